# revision 1
# baseline (speedup 1.0000x reference)
"""GAT (2-layer graph attention network) Bass kernel for 8 trn2 NeuronCores.

Sharding: core c owns node rows [512c, 512c+512). Weights replicated.
Scores are computed in transposed layout [j(partitions), i(free)] so the
aggregation matmul out1T[d', i] = sum_j h_aug[j, d'] * P[j, i] needs no
on-device transposes. The softmax denominator comes from a ones column in
the augmented feature matrix (partition row 64 of the PSUM accumulator).
Large matmuls run in float32r (full PE rate, ~2e-5 rel err).
"""

import os

import numpy as np

N, FIN, HID, H, D1, C = 4096, 512, 256, 4, 64, 64
NCORES = 8
SH = N // NCORES          # 512 local nodes per core
NB = N // 128             # 32 j-chunks
FC = FIN // 128           # 4 fin chunks
KC2 = HID // 128          # 2 hid chunks
NEG = 0.2                 # leaky relu slope
AUG = (D1 + 1) * H        # 260: [ones, h0, ones, h1, ones, h2, ones, h3]

_CACHED = {}


def _make_act_root(alpha=NEG):
    """Patch the neuron ACT tables so Exp computes g(x)=exp(lrelu(x)).

    Bucket entries are [d0,d1,d2,d3,x0,0,0,0] fp32 cubics evaluated as
    y = d0+(x-x0)(d1+(x-x0)(d2+(x-x0)d3)). For exp buckets centered at
    x0<0 we substitute the Taylor cubic of exp(alpha*x) at the same
    center; the alpha contraction makes the cubic far more accurate than
    the original spline tolerance. Verified on HW: max rel err ~1.1e-5.
    """
    import json
    import shutil
    import tempfile

    from neuronxcc.driver.Job import Job
    from neuronxcc.driver.jobs.support.FindActInfo import findActInfoFile

    src_dir = os.path.dirname(findActInfoFile(Job.getPackageDir(), "gen3"))
    dst = tempfile.mkdtemp(prefix="gat_act_root_")
    for f in os.listdir(src_dir):
        shutil.copy(os.path.join(src_dir, f), os.path.join(dst, f))
        os.chmod(os.path.join(dst, f), 0o644)
    for set_name in ("exp_and_others", "natural_log_exp_and_others",
                     "exp_and_friends"):
        meta = json.load(open(os.path.join(dst, f"{set_name}.json")))
        start = meta["func_to_bkt_start_idx"].get("exp")
        if start is None:
            continue
        nxt = [s for s in sorted(meta["func_to_bkt_start_idx"].values())
               if s > start]
        end = nxt[0] if nxt else meta["bkt_entry_cnt"]
        path = os.path.join(dst, f"{set_name}_bkt.bin")
        b = np.fromfile(path, dtype=np.float32).reshape(-1, 8).copy()
        for i in range(start, end):
            x0, d0 = float(b[i, 4]), float(b[i, 0])
            if x0 >= 0 or not np.isfinite(d0) or d0 <= 0:
                continue
            e = np.exp(alpha * x0)
            b[i, 0:4] = [e, alpha * e, alpha * alpha * e / 2.0,
                         alpha ** 3 * e / 6.0]
        b.tofile(path)
    return os.path.join(dst, "act_info.json")


def _build_nc():
    os.environ["BASS_ACT_ROOT_JSON_PATH"] = _make_act_root()
    import concourse.mybir as mybir
    import concourse.tile as tile
    from concourse import bacc

    f32 = mybir.dt.float32
    f32r = mybir.dt.float32r
    bf16 = mybir.dt.bfloat16
    Af = mybir.ActivationFunctionType
    Alu = mybir.AluOpType

    nc = bacc.Bacc("TRN2", target_bir_lowering=False, debug=False,
                   num_devices=NCORES)

    xT_d = nc.dram_tensor("xT", [FIN, N], f32r, kind="ExternalInput").ap()
    xsT_d = nc.dram_tensor("xsT", [FIN, SH], f32r, kind="ExternalInput").ap()
    mT_d = nc.dram_tensor("maskT", [N, SH], bf16, kind="ExternalInput").ap()
    W1e_d = nc.dram_tensor("W1e", [FIN, HID + H], f32r, kind="ExternalInput").ap()
    V1s_d = nc.dram_tensor("V1s", [FIN, H], f32r, kind="ExternalInput").ap()
    W2e_d = nc.dram_tensor("W2e", [HID, C + 1], f32, kind="ExternalInput").ap()
    v2s_d = nc.dram_tensor("v2s", [HID, 1], f32, kind="ExternalInput").ap()
    outT_d = nc.dram_tensor("outT", [C, SH], f32, kind="ExternalOutput").ap()

    with tile.TileContext(nc) as tc:
        with tc.tile_pool(name="persist", bufs=1) as pp:
            h1aug = pp.tile([128, NB, AUG], bf16)
            maskr = pp.tile([128, NB, SH], bf16)
            sdst = pp.tile([128, NB, H], f32)
            ssrcb = pp.tile([128, H, SH], f32)
            ssrow = pp.tile([1, H, SH], f32)
            z1Tl = pp.tile([128, KC2, SH], f32)
            z1Tf = pp.tile([128, KC2, N], f32)
            h2aug = pp.tile([128, NB, D1 + 1], bf16)
            s2dst = pp.tile([128, NB, 1], f32)
            s2srcb = pp.tile([128, SH], f32)
            s2srow = pp.tile([1, SH], f32)
            ones_col = pp.tile([128, 1], f32)
            nc.vector.memset(ones_col[:], 1.0)
            W2sb = pp.tile([128, KC2, C + 1], f32)
            v2sb = pp.tile([128, KC2, 1], f32)

            for kc in range(KC2):
                nc.sync.dma_start(W2sb[:, kc, :], W2e_d[kc * 128:(kc + 1) * 128, :])
                nc.sync.dma_start(v2sb[:, kc, :], v2s_d[kc * 128:(kc + 1) * 128, :])

            # ---------- prep: h1_ext = x @ [W1 | W1.a1_dst], s_src rows ----
            with (tc.tile_pool(name="prep", bufs=1) as prep,
                  tc.tile_pool(name="ppsum", bufs=2, space="PSUM") as ppsum):
                xTt = prep.tile([128, FC, N], f32r)
                xsTt = prep.tile([128, FC, SH], f32r)
                W1et = prep.tile([128, FC, HID + H], f32r)
                V1st = prep.tile([128, FC, H], f32r)
                for fc in range(FC):
                    sl = slice(fc * 128, (fc + 1) * 128)
                    nc.sync.dma_start(xTt[:, fc, :], xT_d[sl, :])
                    nc.sync.dma_start(xsTt[:, fc, :], xsT_d[sl, :])
                    nc.sync.dma_start(W1et[:, fc, :], W1e_d[sl, :])
                    nc.sync.dma_start(V1st[:, fc, :], V1s_d[sl, :])

                # s_src for the local shard, one [1, SH] row per head
                for h in range(H):
                    sps = ppsum.tile([1, SH], f32, tag="sps", bufs=1)
                    for fc in range(FC):
                        nc.tensor.matmul(sps[:], V1st[:, fc, h:h + 1],
                                         xsTt[:, fc, :],
                                         start=(fc == 0), stop=(fc == FC - 1))
                    nc.vector.tensor_copy(ssrow[:, h, :], sps[:])
                    nc.gpsimd.partition_broadcast(ssrcb[:, h, :],
                                                  ssrow[:, h, :])

                # h1_ext per node block; write into the augmented layout
                for nb in range(NB):
                    hp = ppsum.tile([128, HID + H], f32, tag="hp")
                    for fc in range(FC):
                        nc.tensor.matmul(
                            hp[:], xTt[:, fc, nb * 128:(nb + 1) * 128],
                            W1et[:, fc, :],
                            start=(fc == 0), stop=(fc == FC - 1))
                    augv = h1aug[:, nb, :].rearrange("p (h x) -> p h x", x=D1 + 1)
                    nc.vector.tensor_copy(
                        augv[:, :, D1:D1 + 1],
                        ones_col[:].unsqueeze(1).to_broadcast((128, H, 1)))
                    nc.vector.tensor_copy(
                        augv[:, :, 0:D1],
                        hp[:, 0:HID].rearrange("p (h d) -> p h d", h=H))
                    nc.vector.tensor_copy(sdst[:, nb, :], hp[:, HID:HID + H])

            # ---------- layer 1: masked softmax + aggregation --------------
            with tc.tile_pool(name="aggps", bufs=1, space="PSUM") as aggps:
                o1 = aggps.tile([D1 + 1, H, SH], f32)
                with tc.tile_pool(name="work", bufs=4) as wpool:
                    for jc in range(NB):
                        nc.sync.dma_start(maskr[:, jc, :],
                                          mT_d[jc * 128:(jc + 1) * 128, :])
                        pex = wpool.tile([128, H, SH], bf16, tag="pex")
                        for h in range(H):
                            nc.scalar.activation(
                                pex[:, h, :], ssrcb[:, h, :], Af.Exp,
                                bias=sdst[:, jc, h:h + 1])
                        pt = wpool.tile([128, H, SH], bf16, tag="pt")
                        nc.vector.tensor_mul(
                            pt[:], pex[:],
                            maskr[:, jc, :].unsqueeze(1).to_broadcast(
                                (128, H, SH)))
                        for h in range(H):
                            nc.tensor.matmul(
                                o1[:, h, :],
                                h1aug[:, jc, (D1 + 1) * h:(D1 + 1) * (h + 1)],
                                pt[:, h, :],
                                start=(jc == 0), stop=(jc == NB - 1))

                # normalize + ELU -> z1Tl [256(=2x128), SH] transposed layout
                with tc.tile_pool(name="fin1", bufs=1) as fin:
                    for h in range(H):
                        rec = fin.tile([1, SH], f32, tag=f"rec{h}")
                        nc.vector.reciprocal(rec[:], o1[D1:D1 + 1, h, :])
                        recb = fin.tile([D1, SH], f32, tag=f"recb{h}")
                        nc.gpsimd.partition_broadcast(recb[:], rec[:])
                        r0 = (h % 2) * D1
                        nc.vector.tensor_mul(z1Tl[r0:r0 + D1, h // 2, :],
                                             o1[0:D1, h, :], recb[:])
                    for kc in range(KC2):
                        r_ = fin.tile([128, SH], f32, tag="relu")
                        m_ = fin.tile([128, SH], f32, tag="minv")
                        e_ = fin.tile([128, SH], f32, tag="expv")
                        nc.vector.tensor_scalar_max(r_[:], z1Tl[:, kc, :], 0.0)
                        nc.vector.tensor_scalar_min(m_[:], z1Tl[:, kc, :], 0.0)
                        nc.scalar.activation(e_[:], m_[:], Af.Exp, scale=5.0)
                        nc.vector.scalar_tensor_tensor(
                            z1Tl[:, kc, :], e_[:], -1.0, r_[:],
                            op0=Alu.add, op1=Alu.add)

            # ---------- all-gather z1T across the 8 cores -------------------
            with tc.tile_pool(name="dram", bufs=1, space="DRAM") as dpool:
                ag_in = dpool.tile([HID, SH], f32)
                ag_out = dpool.tile([HID * NCORES, SH], f32,
                                    addr_space="Shared")
                for kc in range(KC2):
                    nc.sync.dma_start(ag_in[kc * 128:(kc + 1) * 128, :],
                                      z1Tl[:, kc, :])
                nc.gpsimd.collective_compute(
                    "AllGather", Alu.bypass,
                    replica_groups=[list(range(NCORES))],
                    ins=[ag_in[:].opt()], outs=[ag_out[:].opt()])
                for r in range(NCORES):
                    for kc in range(KC2):
                        src = ag_out[r * HID + kc * 128:
                                     r * HID + (kc + 1) * 128, :]
                        nc.sync.dma_start(z1Tf[:, kc, r * SH:(r + 1) * SH], src)

            # ---------- layer 2 prep: h2, s2_src, s2_dst --------------------
            with tc.tile_pool(name="l2ps", bufs=2, space="PSUM") as l2ps:
                s2p = l2ps.tile([1, SH], f32, tag="s2p")
                for kc in range(KC2):
                    nc.tensor.matmul(s2p[:], v2sb[:, kc, :], z1Tl[:, kc, :],
                                     start=(kc == 0), stop=(kc == KC2 - 1))
                nc.any.tensor_copy(s2srow[:], s2p[:])
                nc.gpsimd.partition_broadcast(s2srcb[:], s2srow[:])
                for nb in range(NB):
                    h2p = l2ps.tile([128, C + 1], f32, tag="h2p")
                    for kc in range(KC2):
                        blk = z1Tf[:, kc, nb * 128:(nb + 1) * 128]
                        nc.tensor.matmul(h2p[:], blk, W2sb[:, kc, :],
                                         start=(kc == 0), stop=(kc == KC2 - 1))
                    nc.vector.tensor_copy(h2aug[:, nb, D1:D1 + 1], ones_col[:])
                    nc.vector.tensor_copy(h2aug[:, nb, 0:D1], h2p[:, 0:C])
                    nc.vector.tensor_copy(s2dst[:, nb, :], h2p[:, C:C + 1])

            # ---------- layer 2: masked softmax + aggregation ---------------
            with tc.tile_pool(name="aggps2", bufs=1, space="PSUM") as aggps2:
                o2 = aggps2.tile([D1 + 1, SH], f32)
                with tc.tile_pool(name="work2", bufs=4) as wpool2:
                    for jc in range(NB):
                        pex = wpool2.tile([128, SH], bf16, tag="pexb")
                        nc.scalar.activation(
                            pex[:], s2srcb[:], Af.Exp,
                            bias=s2dst[:, jc, :])
                        pt = wpool2.tile([128, SH], bf16, tag="ptb")
                        nc.vector.tensor_mul(pt[:], pex[:], maskr[:, jc, :])
                        nc.tensor.matmul(o2[:], h2aug[:, jc, :], pt[:],
                                         start=(jc == 0), stop=(jc == NB - 1))

                with tc.tile_pool(name="fin2", bufs=1) as fin2:
                    rec = fin2.tile([1, SH], f32, tag="rec2")
                    nc.vector.reciprocal(rec[:], o2[D1:D1 + 1, :])
                    recb = fin2.tile([C, SH], f32, tag="recb2")
                    nc.gpsimd.partition_broadcast(recb[:], rec[:])
                    outsb = fin2.tile([C, SH], f32, tag="outsb")
                    nc.vector.tensor_mul(outsb[:], o2[0:D1, :], recb[:])
                    nc.sync.dma_start(outT_d, outsb[:])

    nc.compile()
    return nc


def _get_nc():
    if "nc" not in _CACHED:
        _CACHED["nc"] = _build_nc()
    return _CACHED["nc"]


def _prep_in_maps(x, A, W1, a1_src, a1_dst, W2, a2_src, a2_dst):
    import ml_dtypes
    f = np.float32
    xT = np.ascontiguousarray(x.T).astype(f, copy=False)
    W1r = W1.reshape(FIN, H, D1)
    V1s = np.einsum("fhd,hd->fh", W1r, a1_src).astype(f)
    V1d = np.einsum("fhd,hd->fh", W1r, a1_dst).astype(f)
    W1e = np.ascontiguousarray(np.concatenate([W1, V1d], axis=1)).astype(f, copy=False)
    W2e = np.ascontiguousarray(
        np.concatenate([W2, W2 @ a2_dst.T], axis=1)).astype(f, copy=False)
    v2s = np.ascontiguousarray(W2 @ a2_src.T).astype(f, copy=False)
    in_maps = []
    for c in range(NCORES):
        sl = slice(c * SH, (c + 1) * SH)
        in_maps.append({
            "xT": xT,
            "xsT": np.ascontiguousarray(xT[:, sl]),
            "maskT": np.ascontiguousarray((A[sl, :] > 0).T).astype(
                ml_dtypes.bfloat16),
            "W1e": W1e,
            "V1s": V1s,
            "W2e": W2e,
            "v2s": v2s,
        })
    return in_maps


def kernel(x, A, W1, a1_src, a1_dst, W2, a2_src, a2_dst, _want_results=False):
    from concourse.bass_utils import run_bass_kernel_spmd

    nc = _get_nc()
    in_maps = _prep_in_maps(np.asarray(x), np.asarray(A), np.asarray(W1),
                            np.asarray(a1_src), np.asarray(a1_dst),
                            np.asarray(W2), np.asarray(a2_src),
                            np.asarray(a2_dst))
    trace = bool(int(os.environ.get("GAT_TRACE", "0")))
    res = run_bass_kernel_spmd(nc, in_maps, core_ids=list(range(NCORES)),
                               trace=trace)
    out = np.empty((N, C), np.float32)
    for c in range(NCORES):
        out[c * SH:(c + 1) * SH, :] = res.results[c]["outT"].T
    if _want_results:
        return out, res
    return out



# revision 3
# speedup vs baseline: 1.0050x; 1.0050x over previous
"""GAT (2-layer graph attention network) Bass kernel for 8 trn2 NeuronCores.

Sharding: core c owns node rows [512c, 512c+512). Each core projects only its
own 512 nodes (h = x_own @ W1) and all-gathers the augmented per-head blocks
[h | 1] plus an exp(s_dst)-scaled copy (htil); attention exponentials are
computed per j-chunk in transposed layout [j(partitions), i(free)].

Two per-chunk paths share one PSUM accumulator per head:
  scalar path: P = patchedExp(s_src[i] + s_dst[j]) * mask   (ACT engine)
  MAX path:    P = max(E2s[i]*qd[j], E1s[i]) * mask         (DVE engine),
               using exp(lrelu(t)) = max(exp(t), exp(0.2 t)) with exp(s_dst)
               folded into the stationary htil and exp terms of s_src folded
               into host-precomputed broadcast rows.
The patched ACT exp table computes exp(lrelu(x)); tables that need a true exp
(E1s/E2s/qd/E1d) are host-side folds of the rank-1 score projections
x @ (W1 a1_*). Softmax reciprocals run as ln/exp on the scalar engine:
rec = patchedExp(-5*ln(K*den)) = 1/(K*den). Layer 1 runs as two head-pair
phases so the z1 all-gather of the first pair hides under the second.
"""

import os

import numpy as np

N, FIN, HID, H, D1, C = 4096, 512, 256, 4, 64, 64
NCORES = 8
SH = N // NCORES          # 512 local nodes per core
NB = N // 128             # 32 j-chunks
FC = FIN // 128           # 4 fin chunks
OWN = 4                   # own j-blocks per core
NEG = 0.2
AUGH = D1 + 1             # 65 per head
AUG = AUGH * H            # 260
KREC = 32.0               # reciprocal pre-scale (keeps ln(K*den) in (0, 17))
K_SCALAR = 21             # chunks 0..K-1 per phase take the ACT path
K_SCALAR2 = 32            # layer-2 chunks on the ACT path (rest MAX, unused)

_CACHED = {}


def _make_act_root(alpha=NEG):
    """Patch the neuron ACT tables so Exp computes g(x)=exp(lrelu(x)).

    Bucket entries are [d0,d1,d2,d3,x0,0,0,0] fp32 cubics evaluated as
    y = d0+(x-x0)(d1+(x-x0)(d2+(x-x0)d3)). For exp buckets centered at
    x0<0 we substitute the Taylor cubic of exp(alpha*x) at the same
    center. Ln buckets are untouched.
    """
    import json
    import shutil
    import tempfile

    from neuronxcc.driver.Job import Job
    from neuronxcc.driver.jobs.support.FindActInfo import findActInfoFile

    src_dir = os.path.dirname(findActInfoFile(Job.getPackageDir(), "gen3"))
    dst = tempfile.mkdtemp(prefix="gat_act_root_")
    for f in os.listdir(src_dir):
        shutil.copy(os.path.join(src_dir, f), os.path.join(dst, f))
        os.chmod(os.path.join(dst, f), 0o644)
    for set_name in ("exp_and_others", "natural_log_exp_and_others",
                     "exp_and_friends"):
        meta = json.load(open(os.path.join(dst, f"{set_name}.json")))
        start = meta["func_to_bkt_start_idx"].get("exp")
        if start is None:
            continue
        nxt = [s for s in sorted(meta["func_to_bkt_start_idx"].values())
               if s > start]
        end = nxt[0] if nxt else meta["bkt_entry_cnt"]
        path = os.path.join(dst, f"{set_name}_bkt.bin")
        b = np.fromfile(path, dtype=np.float32).reshape(-1, 8).copy()
        for i in range(start, end):
            x0, d0 = float(b[i, 4]), float(b[i, 0])
            if x0 >= 0 or not np.isfinite(d0) or d0 <= 0:
                continue
            e = np.exp(alpha * x0)
            b[i, 0:4] = [e, alpha * e, alpha * alpha * e / 2.0,
                         alpha ** 3 * e / 6.0]
        b.tofile(path)
    return os.path.join(dst, "act_info.json")


def _build_nc():
    os.environ["BASS_ACT_ROOT_JSON_PATH"] = _make_act_root()
    import concourse.mybir as mybir
    import concourse.tile as tile
    from concourse import bacc

    f32 = mybir.dt.float32
    f32r = mybir.dt.float32r
    bf16 = mybir.dt.bfloat16
    Af = mybir.ActivationFunctionType
    Alu = mybir.AluOpType

    nc = bacc.Bacc("TRN2", target_bir_lowering=False, debug=False,
                   num_devices=NCORES)

    xsT_d = nc.dram_tensor("xsT", [FIN, SH], f32r, kind="ExternalInput").ap()
    mT_d = nc.dram_tensor("maskT", [N, SH], bf16, kind="ExternalInput").ap()
    W1_d = nc.dram_tensor("W1a", [FIN, HID], f32r, kind="ExternalInput").ap()
    ssrcb_d = nc.dram_tensor("ssrcb", [128, H, SH], f32, kind="ExternalInput").ap()
    E1sb_d = nc.dram_tensor("E1sb", [128, H, SH], bf16, kind="ExternalInput").ap()
    E2sb_d = nc.dram_tensor("E2sb", [128, H, SH], bf16, kind="ExternalInput").ap()
    sdst_d = nc.dram_tensor("sdstT", [128, NB, H], f32, kind="ExternalInput").ap()
    qd_d = nc.dram_tensor("qdT", [128, NB, H], f32, kind="ExternalInput").ap()
    E1dl_d = nc.dram_tensor("E1dloc", [128, OWN, H], f32, kind="ExternalInput").ap()
    W2e_d = nc.dram_tensor("W2e", [HID, C + 1], bf16, kind="ExternalInput").ap()
    v2s_d = nc.dram_tensor("v2s", [HID, 1], bf16, kind="ExternalInput").ap()
    outT_d = nc.dram_tensor("outT", [C, SH], f32, kind="ExternalOutput").ap()

    with tile.TileContext(nc) as tc:
        with (tc.tile_pool(name="persist", bufs=1) as pp,
              tc.tile_pool(name="dram", bufs=1, space="DRAM") as dpool):
            # ---------------- persistent SBUF tiles -----------------------
            maskr = pp.tile([128, NB, SH], bf16)
            h1all = pp.tile([128, NB, 2 * AUG], bf16)   # [h1aug | htil]
            ssrcb = pp.tile([128, H, SH], f32)
            E1sb = pp.tile([128, H, SH], bf16)
            E2sb = pp.tile([128, H, SH], bf16)
            sdstT = pp.tile([128, NB, H], f32)
            qdT = pp.tile([128, NB, H], f32)
            E1dloc = pp.tile([128, OWN, H], f32)
            xsTt = pp.tile([128, FC, SH], f32r)
            W1sb = pp.tile([128, FC, HID], f32r)
            h1loc = pp.tile([128, OWN, AUG], bf16)
            htloc = pp.tile([128, OWN, AUG], bf16)
            z1Tl = pp.tile([128, 2, SH], bf16)
            z1Tf = pp.tile([128, 2, N], bf16)
            h2aug = pp.tile([128, NB, AUGH], bf16)
            s2dst = pp.tile([128, NB, 1], f32)
            s2srow = pp.tile([1, SH], f32)
            s2srcb = pp.tile([128, SH], f32)
            W2sb = pp.tile([128, 2, C + 1], bf16)
            v2sb = pp.tile([128, 2, 1], bf16)
            ones_col = pp.tile([128, 1], bf16)
            rec2row = pp.tile([1, SH], f32)

            ag1_in = dpool.tile([OWN * 128, 2 * AUG], bf16)
            ag1_out = dpool.tile([N, 2 * AUG], bf16, addr_space="Shared")
            ag2a_in = dpool.tile([128, SH], bf16)
            ag2a_out = dpool.tile([128 * NCORES, SH], bf16, addr_space="Shared")
            ag2b_in = dpool.tile([128, SH], bf16)
            ag2b_out = dpool.tile([128 * NCORES, SH], bf16, addr_space="Shared")

            # ---------------- input DMAs (phase-A critical first) ---------
            for fc in range(FC):
                nc.sync.dma_start(xsTt[:, fc, :], xsT_d[fc * 128:(fc + 1) * 128, :])
                nc.sync.dma_start(W1sb[:, fc, :], W1_d[fc * 128:(fc + 1) * 128, :])
            nc.sync.dma_start(ssrcb[:], ssrcb_d)
            nc.sync.dma_start(sdstT[:], sdst_d)
            nc.sync.dma_start(E1dloc[:], E1dl_d)
            nc.sync.dma_start(E1sb[:], E1sb_d)
            nc.sync.dma_start(E2sb[:], E2sb_d)
            nc.sync.dma_start(qdT[:], qd_d)
            for kc in range(2):
                nc.sync.dma_start(W2sb[:, kc, :], W2e_d[kc * 128:(kc + 1) * 128, :])
                nc.sync.dma_start(v2sb[:, kc, :], v2s_d[kc * 128:(kc + 1) * 128, :])
            for jc in range(NB):
                nc.sync.dma_start(maskr[:, jc, :],
                                  mT_d[jc * 128:(jc + 1) * 128, :])
            nc.vector.memset(ones_col[:], 1.0)

            # ---------------- local prep: h1aug / htil for own blocks -----
            with tc.tile_pool(name="ppsum", bufs=2, space="PSUM") as ppsum:
                h1v = h1loc[:].rearrange("p k (h x) -> p k h x", x=AUGH)
                nc.vector.tensor_copy(
                    h1v[:, :, :, D1:D1 + 1],
                    ones_col[:].unsqueeze(1).unsqueeze(1).to_broadcast(
                        (128, OWN, H, 1)))
                for k in range(OWN):
                    hp = ppsum.tile([128, HID], f32, tag="hp")
                    for fc in range(FC):
                        nc.tensor.matmul(
                            hp[:], xsTt[:, fc, k * 128:(k + 1) * 128],
                            W1sb[:, fc, :],
                            start=(fc == 0), stop=(fc == FC - 1))
                    nc.vector.tensor_copy(
                        h1v[:, k, :, 0:D1],
                        hp[:].rearrange("p (h d) -> p h d", h=H))
                    for h in range(H):
                        nc.vector.tensor_scalar_mul(
                            htloc[:, k, h * AUGH:(h + 1) * AUGH],
                            h1loc[:, k, h * AUGH:(h + 1) * AUGH],
                            E1dloc[:, k, h:h + 1])
                    nc.sync.dma_start(ag1_in[k * 128:(k + 1) * 128, 0:AUG],
                                      h1loc[:, k, :])
                    nc.sync.dma_start(ag1_in[k * 128:(k + 1) * 128, AUG:2 * AUG],
                                      htloc[:, k, :])

            nc.gpsimd.collective_compute(
                "AllGather", Alu.bypass,
                replica_groups=[list(range(NCORES))],
                ins=[ag1_in[:].opt()], outs=[ag1_out[:].opt()])

            # stream gathered blocks into SBUF (queue in use order)
            for jc in range(NB):
                nc.sync.dma_start(h1all[:, jc, :],
                                  ag1_out[jc * 128:(jc + 1) * 128, :])

            # ---------------- layer 1: two head-phases --------------------
            with tc.tile_pool(name="l1ps", bufs=1, space="PSUM") as l1ps:
                o1A = l1ps.tile([AUGH, 2, SH], f32, tag="o1A")
                o1B = l1ps.tile([AUGH, 2, SH], f32, tag="o1B")

                def l1_phase(ph, o1):
                    hs = [2 * ph, 2 * ph + 1]
                    with tc.tile_pool(name=f"work{ph}", bufs=6) as wp:
                        for t in range(NB):
                            jc = t
                            mb = maskr[:, jc, :].unsqueeze(1).to_broadcast(
                                (128, 2, SH))
                            if t < K_SCALAR:
                                pex = wp.tile([128, 2, SH], bf16, tag="pex")
                                for u, h in enumerate(hs):
                                    nc.scalar.activation(
                                        pex[:, u, :], ssrcb[:, h, :], Af.Exp,
                                        bias=sdstT[:, jc, h:h + 1])
                                pt = wp.tile([128, 2, SH], bf16, tag="pt")
                                nc.vector.tensor_mul(pt[:], pex[:], mb)
                                src, off = pt, 0
                            else:
                                m0 = wp.tile([128, 2, SH], bf16, tag="m0")
                                for u, h in enumerate(hs):
                                    nc.vector.scalar_tensor_tensor(
                                        m0[:, u, :], E2sb[:, h, :],
                                        qdT[:, jc, h:h + 1], E1sb[:, h, :],
                                        op0=Alu.mult, op1=Alu.max)
                                m1 = wp.tile([128, 2, SH], bf16, tag="m1")
                                nc.vector.tensor_mul(m1[:], m0[:], mb)
                                src, off = m1, AUG
                            for u, h in enumerate(hs):
                                nc.tensor.matmul(
                                    o1[:, u, :],
                                    h1all[:, jc,
                                          off + AUGH * h:off + AUGH * (h + 1)],
                                    src[:, u, :],
                                    start=(t == 0), stop=(t == NB - 1))

                def l1_epilogue(ph, o1, ag_in):
                    with tc.tile_pool(name=f"fin{ph}", bufs=1) as fin:
                        lnv = fin.tile([1, 2, SH], f32, tag="lnv")
                        nc.scalar.activation(lnv[:], o1[D1:D1 + 1, :, :],
                                             Af.Ln, scale=KREC)
                        rr = fin.tile([1, 2, SH], f32, tag="rr")
                        nc.scalar.activation(rr[:], lnv[:], Af.Exp, scale=-5.0)
                        zrow = fin.tile([128, SH], f32, tag="zrow")
                        for u in range(2):
                            recb = fin.tile([D1, SH], f32, tag=f"recb{u}")
                            nc.gpsimd.partition_broadcast(recb[:], rr[:, u, :])
                            nc.vector.scalar_tensor_tensor(
                                zrow[u * D1:(u + 1) * D1, :], o1[0:D1, u, :],
                                KREC, recb[:], op0=Alu.mult, op1=Alu.mult)
                        # ELU: max(z,0) + patchedExp(5*min(z,0)) - 1
                        rmax = fin.tile([128, SH], f32, tag="rmax")
                        rmin = fin.tile([128, SH], f32, tag="rmin")
                        ex = fin.tile([128, SH], f32, tag="ex")
                        nc.vector.tensor_scalar_max(rmax[:], zrow[:], 0.0)
                        nc.vector.tensor_scalar_min(rmin[:], zrow[:], 0.0)
                        nc.scalar.activation(ex[:], rmin[:], Af.Exp, scale=5.0)
                        nc.vector.scalar_tensor_tensor(
                            z1Tl[:, ph, :], ex[:], -1.0, rmax[:],
                            op0=Alu.add, op1=Alu.add)
                        nc.sync.dma_start(ag_in[:], z1Tl[:, ph, :])

                l1_phase(0, o1A)
                l1_epilogue(0, o1A, ag2a_in)
                nc.gpsimd.collective_compute(
                    "AllGather", Alu.bypass,
                    replica_groups=[list(range(NCORES))],
                    ins=[ag2a_in[:].opt()], outs=[ag2a_out[:].opt()])
                for r in range(NCORES):
                    nc.sync.dma_start(z1Tf[:, 0, r * SH:(r + 1) * SH],
                                      ag2a_out[r * 128:(r + 1) * 128, :])
                l1_phase(1, o1B)
                l1_epilogue(1, o1B, ag2b_in)

                # s2 score prep uses only local z1; runs before the gpsimd
                # stream blocks on the second all-gather.
                with tc.tile_pool(name="s2ps", bufs=1, space="PSUM") as s2ps:
                    s2p = s2ps.tile([1, SH], f32, tag="s2p")
                    for kc in range(2):
                        nc.tensor.matmul(s2p[:], v2sb[:, kc, :], z1Tl[:, kc, :],
                                         start=(kc == 0), stop=(kc == 1))
                    nc.vector.tensor_copy(s2srow[:], s2p[:])
                    nc.gpsimd.partition_broadcast(s2srcb[:], s2srow[:])

                nc.gpsimd.collective_compute(
                    "AllGather", Alu.bypass,
                    replica_groups=[list(range(NCORES))],
                    ins=[ag2b_in[:].opt()], outs=[ag2b_out[:].opt()])
                for r in range(NCORES):
                    nc.sync.dma_start(z1Tf[:, 1, r * SH:(r + 1) * SH],
                                      ag2b_out[r * 128:(r + 1) * 128, :])

            # ---------------- layer 2 prep --------------------------------
            nc.vector.tensor_copy(
                h2aug[:, :, D1:D1 + 1],
                ones_col[:].unsqueeze(1).to_broadcast((128, NB, 1)))
            with tc.tile_pool(name="l2ps", bufs=2, space="PSUM") as l2ps:
                for nb in range(NB):
                    h2p = l2ps.tile([128, C + 1], f32, tag="h2p")
                    for kc in range(2):
                        nc.tensor.matmul(
                            h2p[:], z1Tf[:, kc, nb * 128:(nb + 1) * 128],
                            W2sb[:, kc, :], start=(kc == 0), stop=(kc == 1))
                    nc.vector.tensor_copy(h2aug[:, nb, 0:D1], h2p[:, 0:C])
                    nc.vector.tensor_copy(s2dst[:, nb, :], h2p[:, C:C + 1])

            # ---------------- layer 2: masked softmax + aggregation -------
            with tc.tile_pool(name="aggps2", bufs=1, space="PSUM") as aggps2:
                o2 = aggps2.tile([AUGH, SH], f32)
                with tc.tile_pool(name="work2", bufs=6) as wp2:
                    for jc in range(NB):
                        pex = wp2.tile([128, SH], bf16, tag="pexb")
                        nc.scalar.activation(pex[:], s2srcb[:], Af.Exp,
                                             bias=s2dst[:, jc, :])
                        pt = wp2.tile([128, SH], bf16, tag="ptb")
                        nc.vector.tensor_mul(pt[:], pex[:], maskr[:, jc, :])
                        nc.tensor.matmul(o2[:], h2aug[:, jc, :], pt[:],
                                         start=(jc == 0), stop=(jc == NB - 1))

                with tc.tile_pool(name="fin2", bufs=1) as fin2:
                    u2 = fin2.tile([1, SH], f32, tag="u2")
                    nc.scalar.activation(u2[:], o2[D1:D1 + 1, :], Af.Ln,
                                         scale=KREC)
                    nc.scalar.activation(rec2row[:], u2[:], Af.Exp, scale=-5.0)
                    recb2 = fin2.tile([C, SH], f32, tag="recb2")
                    nc.gpsimd.partition_broadcast(recb2[:], rec2row[:])
                    outsb = fin2.tile([C, SH], f32, tag="outsb")
                    nc.vector.scalar_tensor_tensor(
                        outsb[:], o2[0:D1, :], KREC, recb2[:],
                        op0=Alu.mult, op1=Alu.mult)
                    nc.sync.dma_start(outT_d, outsb[:])

    nc.compile()
    return nc


def _get_nc():
    if "nc" not in _CACHED:
        _CACHED["nc"] = _build_nc()
    return _CACHED["nc"]


def _prep_in_maps(x, A, W1, a1_src, a1_dst, W2, a2_src, a2_dst):
    import ml_dtypes
    f = np.float32
    bf = ml_dtypes.bfloat16
    xT = np.ascontiguousarray(x.T).astype(f, copy=False)
    W1r = W1.reshape(FIN, H, D1)
    V1s = np.einsum("fhd,hd->fh", W1r, a1_src).astype(f)
    V1d = np.einsum("fhd,hd->fh", W1r, a1_dst).astype(f)
    s_src = (x @ V1s).astype(f)                    # [N, H]
    s_dst = (x @ V1d).astype(f)                    # [N, H]
    E1s = np.exp(s_src)
    E2s = np.exp(NEG * s_src)
    qd_full = np.exp(-(1.0 - NEG) * s_dst)
    E1d_full = np.exp(s_dst)

    def jlay(a):                                   # [N, H] -> [128, NB, H]
        return np.ascontiguousarray(
            a.reshape(NB, 128, H).transpose(1, 0, 2)).astype(f)

    sdstT = jlay(s_dst)
    qdT = jlay(qd_full)
    W2e = np.concatenate([W2, W2 @ a2_dst.T], axis=1).astype(bf)
    v2s = (W2 @ a2_src.T).astype(bf)

    in_maps = []
    for c in range(NCORES):
        sl = slice(c * SH, (c + 1) * SH)

        def ibc(a, dt):                  # [SH, H] rows -> [128, H, SH] bcast
            r = np.ascontiguousarray(a[sl].T)      # [H, SH]
            return np.ascontiguousarray(
                np.broadcast_to(r[None], (128, H, SH))).astype(dt)

        E1dloc = np.ascontiguousarray(
            E1d_full[sl].reshape(OWN, 128, H).transpose(1, 0, 2)).astype(f)
        in_maps.append({
            "xsT": np.ascontiguousarray(xT[:, sl]),
            "maskT": np.ascontiguousarray((A[sl, :] > 0).T).astype(bf),
            "W1a": W1.astype(f, copy=False),
            "ssrcb": ibc(s_src, f),
            "E1sb": ibc(E1s, bf),
            "E2sb": ibc(E2s, bf),
            "sdstT": sdstT,
            "qdT": qdT,
            "E1dloc": E1dloc,
            "W2e": W2e,
            "v2s": v2s,
        })
    return in_maps


def kernel(x, A, W1, a1_src, a1_dst, W2, a2_src, a2_dst, _want_results=False):
    from concourse.bass_utils import run_bass_kernel_spmd

    nc = _get_nc()
    in_maps = _prep_in_maps(np.asarray(x), np.asarray(A), np.asarray(W1),
                            np.asarray(a1_src), np.asarray(a1_dst),
                            np.asarray(W2), np.asarray(a2_src),
                            np.asarray(a2_dst))
    trace = bool(int(os.environ.get("GAT_TRACE", "0")))
    res = run_bass_kernel_spmd(nc, in_maps, core_ids=list(range(NCORES)),
                               trace=trace)
    out = np.empty((N, C), np.float32)
    for c in range(NCORES):
        out[c * SH:(c + 1) * SH, :] = res.results[c]["outT"].T
    if _want_results:
        return out, res
    return out


# revision 4
# speedup vs baseline: 1.1362x; 1.1305x over previous
"""GAT (2-layer graph attention network) Bass kernel for 8 trn2 NeuronCores.

Sharding: core c owns node rows [512c, 512c+512). Each core projects only its
own 512 nodes (h = x_own @ W1) and all-gathers the augmented per-head blocks
[h | 1] plus an exp(s_dst)-scaled copy (htil); attention exponentials are
computed per j-chunk in transposed layout [j(partitions), i(free)].

Two per-chunk paths share one PSUM accumulator per head:
  scalar path: P = patchedExp(s_src[i] + s_dst[j]) * mask   (ACT engine)
  MAX path:    P = max(E2s[i]*qd[j], E1s[i]) * mask         (DVE engine),
               using exp(lrelu(t)) = max(exp(t), exp(0.2 t)) with exp(s_dst)
               folded into the stationary htil and exp terms of s_src folded
               into host-precomputed broadcast rows.
Layer 2 projects h2 for the local node shard only (from local z1 columns) and
all-gathers the small augmented [h2 | 1 | s2_dst] blocks — no full-z1
all-gather. The patched ACT exp table computes exp(lrelu(x)); tables that
need a true exp (E1s/E2s/qd/E1d) are host-side folds of the rank-1 score
projections x @ (W1 a1_*). Softmax reciprocals run as ln/exp on the scalar
engine (rec = patchedExp(-5*ln(K*den)) = 1/(K*den)); Exp is pinned to the
natural_log_exp_and_others table set so Exp/Ln never flip table loads.
"""

import os

import numpy as np

N, FIN, HID, H, D1, C = 4096, 512, 256, 4, 64, 64
NCORES = 8
SH = N // NCORES          # 512 local nodes per core
NB = N // 128             # 32 j-chunks
FC = FIN // 128           # 4 fin chunks
OWN = 4                   # own j-blocks per core
NEG = 0.2
AUGH = D1 + 1             # 65 per head
AUG = AUGH * H            # 260
KREC = 32.0               # reciprocal pre-scale (keeps ln(K*den) in (0, 17))
K_SCALAR = 23             # chunks 0..K-1 per phase take the ACT path

_CACHED = {}


def _make_act_root(alpha=NEG):
    """Patch the neuron ACT tables so Exp computes g(x)=exp(lrelu(x)).

    Bucket entries are [d0,d1,d2,d3,x0,0,0,0] fp32 cubics evaluated as
    y = d0+(x-x0)(d1+(x-x0)(d2+(x-x0)d3)). For exp buckets centered at
    x0<0 we substitute the Taylor cubic of exp(alpha*x) at the same
    center. Ln buckets are untouched.
    """
    import json
    import shutil
    import tempfile

    from neuronxcc.driver.Job import Job
    from neuronxcc.driver.jobs.support.FindActInfo import findActInfoFile

    src_dir = os.path.dirname(findActInfoFile(Job.getPackageDir(), "gen3"))
    dst = tempfile.mkdtemp(prefix="gat_act_root_")
    for f in os.listdir(src_dir):
        shutil.copy(os.path.join(src_dir, f), os.path.join(dst, f))
        os.chmod(os.path.join(dst, f), 0o644)
    for set_name in ("exp_and_others", "natural_log_exp_and_others",
                     "exp_and_friends"):
        meta = json.load(open(os.path.join(dst, f"{set_name}.json")))
        start = meta["func_to_bkt_start_idx"].get("exp")
        if start is None:
            continue
        nxt = [s for s in sorted(meta["func_to_bkt_start_idx"].values())
               if s > start]
        end = nxt[0] if nxt else meta["bkt_entry_cnt"]
        path = os.path.join(dst, f"{set_name}_bkt.bin")
        b = np.fromfile(path, dtype=np.float32).reshape(-1, 8).copy()
        for i in range(start, end):
            x0, d0 = float(b[i, 4]), float(b[i, 0])
            if x0 >= 0 or not np.isfinite(d0) or d0 <= 0:
                continue
            e = np.exp(alpha * x0)
            b[i, 0:4] = [e, alpha * e, alpha * alpha * e / 2.0,
                         alpha ** 3 * e / 6.0]
        b.tofile(path)
    return os.path.join(dst, "act_info.json")


def _pin_exp_table(act_root, mybir, bacc):
    """Make bacc's table-load pass see Exp only in the set that also holds
    Ln, so the whole kernel uses one ACT table load (no Exp<->Ln flips)."""
    import json

    with open(act_root) as f:
        info = json.load(f)
    tables = {}
    for ent in info["act_func_sets"]:
        fns = set()
        for v in ent["act"].keys():
            try:
                fns.add(mybir.ActivationFunctionType.from_pwp(v))
            except Exception:
                pass
        if ent["name"] != "natural_log_exp_and_others":
            fns.discard(mybir.ActivationFunctionType.Exp)
        tables[ent["name"]] = fns
    bacc.get_activation_tables = lambda arch: tables


def _build_nc():
    act_root = _make_act_root()
    os.environ["BASS_ACT_ROOT_JSON_PATH"] = act_root
    import concourse.mybir as mybir
    import concourse.tile as tile
    from concourse import bacc

    _pin_exp_table(act_root, mybir, bacc)

    f32 = mybir.dt.float32
    f32r = mybir.dt.float32r
    bf16 = mybir.dt.bfloat16
    Af = mybir.ActivationFunctionType
    Alu = mybir.AluOpType

    nc = bacc.Bacc("TRN2", target_bir_lowering=False, debug=False,
                   num_devices=NCORES)

    xsT_d = nc.dram_tensor("xsT", [FIN, SH], f32r, kind="ExternalInput").ap()
    mT_d = nc.dram_tensor("maskT", [N, SH], bf16, kind="ExternalInput").ap()
    W1_d = nc.dram_tensor("W1a", [FIN, HID], f32r, kind="ExternalInput").ap()
    ssrcb_d = nc.dram_tensor("ssrcb", [128, H, SH], f32, kind="ExternalInput").ap()
    E1sb_d = nc.dram_tensor("E1sb", [128, H, SH], bf16, kind="ExternalInput").ap()
    E2sb_d = nc.dram_tensor("E2sb", [128, H, SH], bf16, kind="ExternalInput").ap()
    sdst_d = nc.dram_tensor("sdstT", [128, NB, H], f32, kind="ExternalInput").ap()
    qd_d = nc.dram_tensor("qdT", [128, NB, H], f32, kind="ExternalInput").ap()
    E1dl_d = nc.dram_tensor("E1dloc", [128, OWN, H], f32, kind="ExternalInput").ap()
    W2e_d = nc.dram_tensor("W2e", [HID, C + 1], bf16, kind="ExternalInput").ap()
    v2s_d = nc.dram_tensor("v2s", [HID, 1], bf16, kind="ExternalInput").ap()
    outT_d = nc.dram_tensor("outT", [C, SH], f32, kind="ExternalOutput").ap()

    with tile.TileContext(nc) as tc:
        with (tc.tile_pool(name="persist", bufs=1) as pp,
              tc.tile_pool(name="dram", bufs=1, space="DRAM") as dpool):
            # ---------------- persistent SBUF tiles -----------------------
            maskr = pp.tile([128, NB, SH], bf16)
            h1all = pp.tile([128, NB, 2 * AUG], bf16)   # [h1aug | htil]
            ssrcb = pp.tile([128, H, SH], f32)
            E1sb = pp.tile([128, H, SH], bf16)
            E2sb = pp.tile([128, H, SH], bf16)
            sdstT = pp.tile([128, NB, H], f32)
            qdT = pp.tile([128, NB, H], f32)
            E1dloc = pp.tile([128, OWN, H], f32)
            xsTt = pp.tile([128, FC, SH], f32r)
            W1sb = pp.tile([128, FC, HID], f32r)
            h1loc = pp.tile([128, OWN, AUG], bf16)
            htloc = pp.tile([128, OWN, AUG], bf16)
            z1Tl = pp.tile([128, 2, SH], bf16)
            h2l = pp.tile([128, OWN, C + 2], bf16)      # [h2 | 1 | s2dst]
            h2all = pp.tile([128, NB, C + 2], bf16)
            s2dst = pp.tile([128, NB, 1], f32)
            s2srow = pp.tile([1, SH], f32)
            s2srcb = pp.tile([128, SH], f32)
            W2sb = pp.tile([128, 2, C + 1], bf16)
            v2sb = pp.tile([128, 2, 1], bf16)
            ones_col = pp.tile([128, 1], bf16)
            rec2row = pp.tile([1, SH], f32)

            ag1_in = dpool.tile([OWN * 128, 2 * AUG], bf16)
            ag1_out = dpool.tile([N, 2 * AUG], bf16, addr_space="Shared")
            ag3_in = dpool.tile([OWN * 128, C + 2], bf16)
            ag3_out = dpool.tile([N, C + 2], bf16, addr_space="Shared")

            # ---------------- AG1-critical input DMAs ---------------------
            for fc in range(FC):
                nc.sync.dma_start(xsTt[:, fc, :], xsT_d[fc * 128:(fc + 1) * 128, :])
                nc.sync.dma_start(W1sb[:, fc, :], W1_d[fc * 128:(fc + 1) * 128, :])
            nc.sync.dma_start(E1dloc[:], E1dl_d)
            nc.vector.memset(ones_col[:], 1.0)

            # ---------------- local prep: h1aug / htil for own blocks -----
            with tc.tile_pool(name="ppsum", bufs=2, space="PSUM") as ppsum:
                h1v = h1loc[:].rearrange("p k (h x) -> p k h x", x=AUGH)
                nc.vector.tensor_copy(
                    h1v[:, :, :, D1:D1 + 1],
                    ones_col[:].unsqueeze(1).unsqueeze(1).to_broadcast(
                        (128, OWN, H, 1)))
                for k in range(OWN):
                    hp = ppsum.tile([128, HID], f32, tag="hp")
                    for fc in range(FC):
                        nc.tensor.matmul(
                            hp[:], xsTt[:, fc, k * 128:(k + 1) * 128],
                            W1sb[:, fc, :],
                            start=(fc == 0), stop=(fc == FC - 1))
                    nc.vector.tensor_copy(
                        h1v[:, k, :, 0:D1],
                        hp[:].rearrange("p (h d) -> p h d", h=H))
                    for h in range(H):
                        nc.vector.tensor_scalar_mul(
                            htloc[:, k, h * AUGH:(h + 1) * AUGH],
                            h1loc[:, k, h * AUGH:(h + 1) * AUGH],
                            E1dloc[:, k, h:h + 1])
                    nc.sync.dma_start(ag1_in[k * 128:(k + 1) * 128, 0:AUG],
                                      h1loc[:, k, :])
                    nc.sync.dma_start(ag1_in[k * 128:(k + 1) * 128, AUG:2 * AUG],
                                      htloc[:, k, :])

            nc.gpsimd.collective_compute(
                "AllGather", Alu.bypass,
                replica_groups=[list(range(NCORES))],
                ins=[ag1_in[:].opt()], outs=[ag1_out[:].opt()])

            # ---------------- deferred input DMAs (stream under AG1) ------
            nc.sync.dma_start(ssrcb[:], ssrcb_d)
            nc.sync.dma_start(sdstT[:], sdst_d)
            nc.sync.dma_start(E1sb[:], E1sb_d)
            nc.sync.dma_start(E2sb[:], E2sb_d)
            nc.sync.dma_start(qdT[:], qd_d)
            for kc in range(2):
                nc.sync.dma_start(W2sb[:, kc, :], W2e_d[kc * 128:(kc + 1) * 128, :])
                nc.sync.dma_start(v2sb[:, kc, :], v2s_d[kc * 128:(kc + 1) * 128, :])
            for jc in range(NB):
                nc.sync.dma_start(maskr[:, jc, :],
                                  mT_d[jc * 128:(jc + 1) * 128, :])
            for jc in range(NB):
                nc.sync.dma_start(h1all[:, jc, :],
                                  ag1_out[jc * 128:(jc + 1) * 128, :])

            # ---------------- layer 1: two head-phases --------------------
            with tc.tile_pool(name="l1ps", bufs=1, space="PSUM") as l1ps:
                o1A = l1ps.tile([AUGH, 2, SH], f32, tag="o1A")
                o1B = l1ps.tile([AUGH, 2, SH], f32, tag="o1B")

                def l1_phase(ph, o1):
                    hs = [2 * ph, 2 * ph + 1]
                    with tc.tile_pool(name=f"work{ph}", bufs=8) as wp:
                        for t in range(NB):
                            jc = t
                            mb = maskr[:, jc, :].unsqueeze(1).to_broadcast(
                                (128, 2, SH))
                            if t < K_SCALAR:
                                pex = wp.tile([128, 2, SH], bf16, tag="e0")
                                for u, h in enumerate(hs):
                                    nc.scalar.activation(
                                        pex[:, u, :], ssrcb[:, h, :], Af.Exp,
                                        bias=sdstT[:, jc, h:h + 1])
                                pt = wp.tile([128, 2, SH], bf16, tag="e2")
                                nc.vector.tensor_mul(pt[:], pex[:], mb)
                                src, off = pt, 0
                            else:
                                t0 = wp.tile([128, 2, SH], bf16, tag="e0")
                                for u, h in enumerate(hs):
                                    nc.vector.tensor_scalar_mul(
                                        t0[:, u, :], E2sb[:, h, :],
                                        qdT[:, jc, h:h + 1])
                                t1 = wp.tile([128, 2, SH], bf16, tag="e1")
                                for u, h in enumerate(hs):
                                    nc.vector.tensor_max(
                                        t1[:, u, :], t0[:, u, :], E1sb[:, h, :])
                                m1 = wp.tile([128, 2, SH], bf16, tag="e2")
                                nc.vector.tensor_mul(m1[:], t1[:], mb)
                                src, off = m1, AUG
                            for u, h in enumerate(hs):
                                nc.tensor.matmul(
                                    o1[:, u, :],
                                    h1all[:, jc,
                                          off + AUGH * h:off + AUGH * (h + 1)],
                                    src[:, u, :],
                                    start=(t == 0), stop=(t == NB - 1))

                def l1_epilogue(ph, o1):
                    with tc.tile_pool(name=f"fin{ph}", bufs=1) as fin:
                        lnv = fin.tile([1, 2, SH], f32, tag="lnv")
                        nc.scalar.activation(lnv[:], o1[D1:D1 + 1, :, :],
                                             Af.Ln, scale=KREC)
                        rr = fin.tile([1, 2, SH], f32, tag="rr")
                        nc.scalar.activation(rr[:], lnv[:], Af.Exp, scale=-5.0)
                        zrow = fin.tile([128, SH], f32, tag="zrow")
                        for u in range(2):
                            recb = fin.tile([D1, SH], f32, tag=f"recb{u}")
                            nc.gpsimd.partition_broadcast(recb[:], rr[:, u, :])
                            nc.vector.scalar_tensor_tensor(
                                zrow[u * D1:(u + 1) * D1, :], o1[0:D1, u, :],
                                KREC, recb[:], op0=Alu.mult, op1=Alu.mult)
                        # ELU: max(z,0) + patchedExp(5*min(z,0)) - 1
                        rmax = fin.tile([128, SH], f32, tag="rmax")
                        rmin = fin.tile([128, SH], f32, tag="rmin")
                        ex = fin.tile([128, SH], f32, tag="ex")
                        nc.vector.tensor_scalar_max(rmax[:], zrow[:], 0.0)
                        nc.vector.tensor_scalar_min(rmin[:], zrow[:], 0.0)
                        nc.scalar.activation(ex[:], rmin[:], Af.Exp, scale=5.0)
                        nc.vector.scalar_tensor_tensor(
                            z1Tl[:, ph, :], ex[:], -1.0, rmax[:],
                            op0=Alu.add, op1=Alu.add)

                l1_phase(0, o1A)
                l1_epilogue(0, o1A)
                l1_phase(1, o1B)
                l1_epilogue(1, o1B)

            # ---------------- layer 2: local h2 projection + all-gather ---
            with tc.tile_pool(name="s2ps", bufs=2, space="PSUM") as s2ps:
                s2p = s2ps.tile([1, SH], f32, tag="s2p", bufs=1)
                for kc in range(2):
                    nc.tensor.matmul(s2p[:], v2sb[:, kc, :], z1Tl[:, kc, :],
                                     start=(kc == 0), stop=(kc == 1))
                nc.vector.tensor_copy(s2srow[:], s2p[:])
                nc.gpsimd.partition_broadcast(s2srcb[:], s2srow[:])

                nc.vector.tensor_copy(
                    h2l[:, :, C:C + 1],
                    ones_col[:].unsqueeze(1).to_broadcast((128, OWN, 1)))
                for k in range(OWN):
                    h2p = s2ps.tile([128, C + 1], f32, tag="h2p")
                    for kc in range(2):
                        nc.tensor.matmul(
                            h2p[:], z1Tl[:, kc, k * 128:(k + 1) * 128],
                            W2sb[:, kc, :], start=(kc == 0), stop=(kc == 1))
                    nc.vector.tensor_copy(h2l[:, k, 0:C], h2p[:, 0:C])
                    nc.vector.tensor_copy(h2l[:, k, C + 1:C + 2],
                                          h2p[:, C:C + 1])
                    nc.sync.dma_start(ag3_in[k * 128:(k + 1) * 128, :],
                                      h2l[:, k, :])

            nc.gpsimd.collective_compute(
                "AllGather", Alu.bypass,
                replica_groups=[list(range(NCORES))],
                ins=[ag3_in[:].opt()], outs=[ag3_out[:].opt()])
            for jc in range(NB):
                nc.sync.dma_start(h2all[:, jc, :],
                                  ag3_out[jc * 128:(jc + 1) * 128, :])
            nc.vector.tensor_copy(s2dst[:], h2all[:, :, C + 1:C + 2])

            # ---------------- layer 2: masked softmax + aggregation -------
            with tc.tile_pool(name="aggps2", bufs=1, space="PSUM") as aggps2:
                o2 = aggps2.tile([AUGH, SH], f32)
                with tc.tile_pool(name="work2", bufs=6) as wp2:
                    for jc in range(NB):
                        pex = wp2.tile([128, SH], bf16, tag="pexb")
                        nc.scalar.activation(pex[:], s2srcb[:], Af.Exp,
                                             bias=s2dst[:, jc, :])
                        pt = wp2.tile([128, SH], bf16, tag="ptb")
                        nc.vector.tensor_mul(pt[:], pex[:], maskr[:, jc, :])
                        nc.tensor.matmul(o2[:], h2all[:, jc, 0:C + 1], pt[:],
                                         start=(jc == 0), stop=(jc == NB - 1))

                with tc.tile_pool(name="fin2", bufs=1) as fin2:
                    u2 = fin2.tile([1, SH], f32, tag="u2")
                    nc.scalar.activation(u2[:], o2[D1:D1 + 1, :], Af.Ln,
                                         scale=KREC)
                    nc.scalar.activation(rec2row[:], u2[:], Af.Exp, scale=-5.0)
                    recb2 = fin2.tile([C, SH], f32, tag="recb2")
                    nc.gpsimd.partition_broadcast(recb2[:], rec2row[:])
                    outsb = fin2.tile([C, SH], f32, tag="outsb")
                    nc.vector.scalar_tensor_tensor(
                        outsb[:], o2[0:D1, :], KREC, recb2[:],
                        op0=Alu.mult, op1=Alu.mult)
                    nc.sync.dma_start(outT_d, outsb[:])

    nc.compile()
    return nc


def _get_nc():
    if "nc" not in _CACHED:
        _CACHED["nc"] = _build_nc()
    return _CACHED["nc"]


def _prep_in_maps(x, A, W1, a1_src, a1_dst, W2, a2_src, a2_dst):
    import ml_dtypes
    f = np.float32
    bf = ml_dtypes.bfloat16
    xT = np.ascontiguousarray(x.T).astype(f, copy=False)
    W1r = W1.reshape(FIN, H, D1)
    V1s = np.einsum("fhd,hd->fh", W1r, a1_src).astype(f)
    V1d = np.einsum("fhd,hd->fh", W1r, a1_dst).astype(f)
    s_src = (x @ V1s).astype(f)                    # [N, H]
    s_dst = (x @ V1d).astype(f)                    # [N, H]
    E1s = np.exp(s_src)
    E2s = np.exp(NEG * s_src)
    qd_full = np.exp(-(1.0 - NEG) * s_dst)
    E1d_full = np.exp(s_dst)

    def jlay(a):                                   # [N, H] -> [128, NB, H]
        return np.ascontiguousarray(
            a.reshape(NB, 128, H).transpose(1, 0, 2)).astype(f)

    sdstT = jlay(s_dst)
    qdT = jlay(qd_full)
    W2e = np.concatenate([W2, W2 @ a2_dst.T], axis=1).astype(bf)
    v2s = (W2 @ a2_src.T).astype(bf)

    in_maps = []
    for c in range(NCORES):
        sl = slice(c * SH, (c + 1) * SH)

        def ibc(a, dt):                  # [SH, H] rows -> [128, H, SH] bcast
            r = np.ascontiguousarray(a[sl].T)      # [H, SH]
            return np.ascontiguousarray(
                np.broadcast_to(r[None], (128, H, SH))).astype(dt)

        E1dloc = np.ascontiguousarray(
            E1d_full[sl].reshape(OWN, 128, H).transpose(1, 0, 2)).astype(f)
        in_maps.append({
            "xsT": np.ascontiguousarray(xT[:, sl]),
            "maskT": np.ascontiguousarray((A[sl, :] > 0).T).astype(bf),
            "W1a": W1.astype(f, copy=False),
            "ssrcb": ibc(s_src, f),
            "E1sb": ibc(E1s, bf),
            "E2sb": ibc(E2s, bf),
            "sdstT": sdstT,
            "qdT": qdT,
            "E1dloc": E1dloc,
            "W2e": W2e,
            "v2s": v2s,
        })
    return in_maps


def kernel(x, A, W1, a1_src, a1_dst, W2, a2_src, a2_dst, _want_results=False):
    from concourse.bass_utils import run_bass_kernel_spmd

    nc = _get_nc()
    in_maps = _prep_in_maps(np.asarray(x), np.asarray(A), np.asarray(W1),
                            np.asarray(a1_src), np.asarray(a1_dst),
                            np.asarray(W2), np.asarray(a2_src),
                            np.asarray(a2_dst))
    trace = bool(int(os.environ.get("GAT_TRACE", "0")))
    res = run_bass_kernel_spmd(nc, in_maps, core_ids=list(range(NCORES)),
                               trace=trace)
    out = np.empty((N, C), np.float32)
    for c in range(NCORES):
        out[c * SH:(c + 1) * SH, :] = res.results[c]["outT"].T
    if _want_results:
        return out, res
    return out


# revision 5
# speedup vs baseline: 1.3030x; 1.1468x over previous
"""GAT (2-layer graph attention network) Bass kernel for 8 trn2 NeuronCores.

Sharding: core c owns node rows [512c, 512c+512). Each core projects only its
own 512 nodes (h = x_own @ W1) and all-gathers the augmented per-head blocks;
attention exponentials are computed per j-chunk in transposed layout
[j(partitions), i(free)].

Per-chunk paths (chosen by jc % 4, so the all-gather payload is one block per
chunk) share one PSUM accumulator per head:
  jc%4<3 (ACT path): P = patchedExp(s_src[i] + s_dst[j]) * mask
  jc%4=3 (MAX path): P = max(E2s[i]*qd[j], E1s[i])*E1d[j] * mask
               via exp(lrelu(t)) = max(exp(t), exp(0.2 t)), with exp(s_dst)
               folded into the gathered stationary block (htil).
Layer 2 projects h2 for the local node shard only (from local z1 columns) and
all-gathers the small augmented [h2 | 1 | s2_dst] blocks. The patched ACT exp
table computes exp(lrelu(x)); tables needing a true exp (E1s/E2s/qd/E1d) are
host-side folds of the rank-1 score projections x @ (W1 a1_*). Softmax
reciprocals run as ln/exp on the scalar engine (rec = patchedExp(-5*ln(K*den))
= 1/(K*den)); Exp is pinned to the natural_log_exp_and_others table set so
Exp/Ln share one table load. Both layer-1 phases are emitted before their
epilogues so no in-order engine stream stalls on a PSUM-gated epilogue op
between phases.
"""

import os

import numpy as np

N, FIN, HID, H, D1, C = 4096, 512, 256, 4, 64, 64
NCORES = 8
SH = N // NCORES          # 512 local nodes per core
NB = N // 128             # 32 j-chunks
FC = FIN // 128           # 4 fin chunks
OWN = 4                   # own j-blocks per core
NEG = 0.2
AUGH = D1 + 1             # 65 per head
AUG = AUGH * H            # 260
KREC = 32.0               # reciprocal pre-scale (keeps ln(K*den) in (0, 17))

_CACHED = {}


def _make_act_root(alpha=NEG):
    """Patch the neuron ACT tables so Exp computes g(x)=exp(lrelu(x)).

    Bucket entries are [d0,d1,d2,d3,x0,0,0,0] fp32 cubics evaluated as
    y = d0+(x-x0)(d1+(x-x0)(d2+(x-x0)d3)). For exp buckets centered at
    x0<0 we substitute the Taylor cubic of exp(alpha*x) at the same
    center. Ln buckets are untouched.
    """
    import json
    import shutil
    import tempfile

    from neuronxcc.driver.Job import Job
    from neuronxcc.driver.jobs.support.FindActInfo import findActInfoFile

    src_dir = os.path.dirname(findActInfoFile(Job.getPackageDir(), "gen3"))
    dst = tempfile.mkdtemp(prefix="gat_act_root_")
    for f in os.listdir(src_dir):
        shutil.copy(os.path.join(src_dir, f), os.path.join(dst, f))
        os.chmod(os.path.join(dst, f), 0o644)
    for set_name in ("exp_and_others", "natural_log_exp_and_others",
                     "exp_and_friends"):
        meta = json.load(open(os.path.join(dst, f"{set_name}.json")))
        start = meta["func_to_bkt_start_idx"].get("exp")
        if start is None:
            continue
        nxt = [s for s in sorted(meta["func_to_bkt_start_idx"].values())
               if s > start]
        end = nxt[0] if nxt else meta["bkt_entry_cnt"]
        path = os.path.join(dst, f"{set_name}_bkt.bin")
        b = np.fromfile(path, dtype=np.float32).reshape(-1, 8).copy()
        for i in range(start, end):
            x0, d0 = float(b[i, 4]), float(b[i, 0])
            if x0 >= 0 or not np.isfinite(d0) or d0 <= 0:
                continue
            e = np.exp(alpha * x0)
            b[i, 0:4] = [e, alpha * e, alpha * alpha * e / 2.0,
                         alpha ** 3 * e / 6.0]
        b.tofile(path)
    return os.path.join(dst, "act_info.json")


def _pin_exp_table(act_root, mybir, bacc):
    """Make bacc's table-load pass see Exp only in the set that also holds
    Ln, so the whole kernel uses one ACT table load (no Exp<->Ln flips)."""
    import json

    with open(act_root) as f:
        info = json.load(f)
    tables = {}
    for ent in info["act_func_sets"]:
        fns = set()
        for v in ent["act"].keys():
            try:
                fns.add(mybir.ActivationFunctionType.from_pwp(v))
            except Exception:
                pass
        if ent["name"] != "natural_log_exp_and_others":
            fns.discard(mybir.ActivationFunctionType.Exp)
        tables[ent["name"]] = fns
    bacc.get_activation_tables = lambda arch: tables


def _build_nc():
    act_root = _make_act_root()
    os.environ["BASS_ACT_ROOT_JSON_PATH"] = act_root
    import concourse.mybir as mybir
    import concourse.tile as tile
    from concourse import bacc

    _pin_exp_table(act_root, mybir, bacc)

    f32 = mybir.dt.float32
    f32r = mybir.dt.float32r
    bf16 = mybir.dt.bfloat16
    Af = mybir.ActivationFunctionType
    Alu = mybir.AluOpType

    nc = bacc.Bacc("TRN2", target_bir_lowering=False, debug=False,
                   num_devices=NCORES)

    xsT_d = nc.dram_tensor("xsT", [FIN, SH], f32r, kind="ExternalInput").ap()
    mT_d = nc.dram_tensor("maskT", [N, SH], bf16, kind="ExternalInput").ap()
    W1_d = nc.dram_tensor("W1a", [FIN, HID], f32r, kind="ExternalInput").ap()
    ssrcb_d = nc.dram_tensor("ssrcb", [128, H, SH], bf16, kind="ExternalInput").ap()
    E1sb_d = nc.dram_tensor("E1sb", [128, H, SH], bf16, kind="ExternalInput").ap()
    E2sb_d = nc.dram_tensor("E2sb", [128, H, SH], bf16, kind="ExternalInput").ap()
    sdst_d = nc.dram_tensor("sdstT", [128, NB, H], f32, kind="ExternalInput").ap()
    qd_d = nc.dram_tensor("qdT", [128, NB, H], f32, kind="ExternalInput").ap()
    E1dl_d = nc.dram_tensor("E1dloc", [128, 1, H], f32, kind="ExternalInput").ap()
    W2e_d = nc.dram_tensor("W2e", [HID, C + 1], bf16, kind="ExternalInput").ap()
    v2s_d = nc.dram_tensor("v2s", [HID, 1], bf16, kind="ExternalInput").ap()
    outT_d = nc.dram_tensor("outT", [C, SH], f32, kind="ExternalOutput").ap()

    with tile.TileContext(nc) as tc:
        with (tc.tile_pool(name="persist", bufs=1) as pp,
              tc.tile_pool(name="dram", bufs=1, space="DRAM") as dpool):
            # ---------------- persistent SBUF tiles -----------------------
            maskr = pp.tile([128, NB, SH], bf16)
            h1all = pp.tile([128, NB, AUG], bf16)
            ssrcb = pp.tile([128, H, SH], bf16)
            E1sb = pp.tile([128, H, SH], bf16)
            E2sb = pp.tile([128, H, SH], bf16)
            sdstT = pp.tile([128, NB, H], f32)
            qdT = pp.tile([128, NB, H], f32)
            E1dloc = pp.tile([128, 1, H], f32)
            xsTt = pp.tile([128, FC, SH], f32r)
            W1sb = pp.tile([128, FC, HID], f32r)
            h1loc = pp.tile([128, OWN, AUG], bf16)
            htloc = pp.tile([128, 1, AUG], bf16)
            z1Tl = pp.tile([128, 2, SH], bf16)
            h2l = pp.tile([128, OWN, C + 2], bf16)      # [h2 | 1 | s2dst]
            h2all = pp.tile([128, NB, C + 2], bf16)
            s2dst = pp.tile([128, NB, 1], f32)
            s2srow = pp.tile([1, SH], f32)
            s2srcb = pp.tile([128, SH], f32)
            W2sb = pp.tile([128, 2, C + 1], bf16)
            v2sb = pp.tile([128, 2, 1], bf16)
            ones_col = pp.tile([128, 1], bf16)
            rec2row = pp.tile([1, SH], f32)

            ag1_in = dpool.tile([OWN * 128, AUG], bf16)
            ag1_out = dpool.tile([N, AUG], bf16, addr_space="Shared")
            ag3_in = dpool.tile([OWN * 128, C + 2], bf16)
            ag3_out = dpool.tile([N, C + 2], bf16, addr_space="Shared")

            # ---------------- AG1-critical input DMAs ---------------------
            for fc in range(FC):
                nc.sync.dma_start(xsTt[:, fc, :], xsT_d[fc * 128:(fc + 1) * 128, :])
                nc.sync.dma_start(W1sb[:, fc, :], W1_d[fc * 128:(fc + 1) * 128, :])
            nc.sync.dma_start(E1dloc[:], E1dl_d)
            nc.vector.memset(ones_col[:], 1.0)

            # ---------------- local prep: h1aug (+ htil for k=3) ----------
            with tc.tile_pool(name="ppsum", bufs=2, space="PSUM") as ppsum:
                h1v = h1loc[:].rearrange("p k (h x) -> p k h x", x=AUGH)
                nc.vector.tensor_copy(
                    h1v[:, :, :, D1:D1 + 1],
                    ones_col[:].unsqueeze(1).unsqueeze(1).to_broadcast(
                        (128, OWN, H, 1)))
                for k in range(OWN):
                    hp = ppsum.tile([128, HID], f32, tag="hp")
                    for fc in range(FC):
                        nc.tensor.matmul(
                            hp[:], xsTt[:, fc, k * 128:(k + 1) * 128],
                            W1sb[:, fc, :],
                            start=(fc == 0), stop=(fc == FC - 1))
                    nc.vector.tensor_copy(
                        h1v[:, k, :, 0:D1],
                        hp[:].rearrange("p (h d) -> p h d", h=H))
                    if k == OWN - 1:
                        for h in range(H):
                            nc.vector.tensor_scalar_mul(
                                htloc[:, 0, h * AUGH:(h + 1) * AUGH],
                                h1loc[:, k, h * AUGH:(h + 1) * AUGH],
                                E1dloc[:, 0, h:h + 1])
                        nc.sync.dma_start(ag1_in[k * 128:(k + 1) * 128, :],
                                          htloc[:, 0, :])
                    else:
                        nc.sync.dma_start(ag1_in[k * 128:(k + 1) * 128, :],
                                          h1loc[:, k, :])

            nc.gpsimd.collective_compute(
                "AllGather", Alu.bypass,
                replica_groups=[list(range(NCORES))],
                ins=[ag1_in[:].opt()], outs=[ag1_out[:].opt()])

            # stream score tables during the all-gather
            nc.sync.dma_start(ssrcb[:], ssrcb_d)
            nc.sync.dma_start(sdstT[:], sdst_d)
            nc.sync.dma_start(E1sb[:], E1sb_d)
            nc.sync.dma_start(E2sb[:], E2sb_d)
            nc.sync.dma_start(qdT[:], qd_d)

            # These block the sync stream until AG1 completes, which also
            # holds the (large) mask/W2 loads off HBM during the collective.
            ag1v = ag1_out[:].rearrange("(jc p) c -> p jc c", p=128)
            nc.sync.dma_start(h1all[:, 0:16, :], ag1v[:, 0:16, :])
            nc.sync.dma_start(h1all[:, 16:NB, :], ag1v[:, 16:NB, :])
            for jc in range(NB):
                nc.sync.dma_start(maskr[:, jc, :],
                                  mT_d[jc * 128:(jc + 1) * 128, :])
            for kc in range(2):
                nc.sync.dma_start(W2sb[:, kc, :], W2e_d[kc * 128:(kc + 1) * 128, :])
                nc.sync.dma_start(v2sb[:, kc, :], v2s_d[kc * 128:(kc + 1) * 128, :])

            # ---------------- layer 1: two head-phases --------------------
            with tc.tile_pool(name="l1ps", bufs=1, space="PSUM") as l1ps:
                o1A = l1ps.tile([AUGH, 2, SH], f32, tag="o1A")
                o1B = l1ps.tile([AUGH, 2, SH], f32, tag="o1B")

                with tc.tile_pool(name="work", bufs=8) as wp:
                    for ph, o1 in ((0, o1A), (1, o1B)):
                        hs = [2 * ph, 2 * ph + 1]
                        for t in range(NB):
                            jc = t
                            mb = maskr[:, jc, :].unsqueeze(1).to_broadcast(
                                (128, 2, SH))
                            if t % 4 != 3:
                                pex = wp.tile([128, 2, SH], bf16, tag="e0")
                                for u, h in enumerate(hs):
                                    nc.scalar.activation(
                                        pex[:, u, :], ssrcb[:, h, :], Af.Exp,
                                        bias=sdstT[:, jc, h:h + 1])
                                pt = wp.tile([128, 2, SH], bf16, tag="e2")
                                nc.vector.tensor_mul(pt[:], pex[:], mb)
                                src = pt
                            else:
                                t0 = wp.tile([128, 2, SH], bf16, tag="e0")
                                for u, h in enumerate(hs):
                                    nc.vector.tensor_scalar_mul(
                                        t0[:, u, :], E2sb[:, h, :],
                                        qdT[:, jc, h:h + 1])
                                t1 = wp.tile([128, 2, SH], bf16, tag="e1")
                                for u, h in enumerate(hs):
                                    nc.vector.tensor_max(
                                        t1[:, u, :], t0[:, u, :], E1sb[:, h, :])
                                m1 = wp.tile([128, 2, SH], bf16, tag="e2")
                                nc.vector.tensor_mul(m1[:], t1[:], mb)
                                src = m1
                            for u, h in enumerate(hs):
                                nc.tensor.matmul(
                                    o1[:, u, :],
                                    h1all[:, jc, AUGH * h:AUGH * (h + 1)],
                                    src[:, u, :],
                                    start=(t == 0), stop=(t == NB - 1))

                for ph, o1 in ((0, o1A), (1, o1B)):
                    with tc.tile_pool(name=f"fin{ph}", bufs=1) as fin:
                        lnv = fin.tile([1, 2, SH], f32, tag="lnv")
                        nc.scalar.activation(lnv[:], o1[D1:D1 + 1, :, :],
                                             Af.Ln, scale=KREC)
                        rr = fin.tile([1, 2, SH], f32, tag="rr")
                        nc.scalar.activation(rr[:], lnv[:], Af.Exp, scale=-5.0)
                        zrow = fin.tile([128, SH], f32, tag="zrow")
                        for u in range(2):
                            recb = fin.tile([D1, SH], f32, tag=f"recb{u}")
                            nc.gpsimd.partition_broadcast(recb[:], rr[:, u, :])
                            nc.vector.scalar_tensor_tensor(
                                zrow[u * D1:(u + 1) * D1, :], o1[0:D1, u, :],
                                KREC, recb[:], op0=Alu.mult, op1=Alu.mult)
                        # ELU: max(z,0) + patchedExp(5*min(z,0)) - 1
                        rmax = fin.tile([128, SH], f32, tag="rmax")
                        rmin = fin.tile([128, SH], f32, tag="rmin")
                        ex = fin.tile([128, SH], f32, tag="ex")
                        nc.vector.tensor_scalar_max(rmax[:], zrow[:], 0.0)
                        nc.vector.tensor_scalar_min(rmin[:], zrow[:], 0.0)
                        nc.scalar.activation(ex[:], rmin[:], Af.Exp, scale=5.0)
                        nc.vector.scalar_tensor_tensor(
                            z1Tl[:, ph, :], ex[:], -1.0, rmax[:],
                            op0=Alu.add, op1=Alu.add)

            # ---------------- layer 2: local h2 projection + all-gather ---
            with tc.tile_pool(name="s2ps", bufs=2, space="PSUM") as s2ps:
                s2p = s2ps.tile([1, SH], f32, tag="s2p", bufs=1)
                for kc in range(2):
                    nc.tensor.matmul(s2p[:], v2sb[:, kc, :], z1Tl[:, kc, :],
                                     start=(kc == 0), stop=(kc == 1))
                nc.vector.tensor_copy(s2srow[:], s2p[:])
                nc.gpsimd.partition_broadcast(s2srcb[:], s2srow[:])

                nc.vector.tensor_copy(
                    h2l[:, :, C:C + 1],
                    ones_col[:].unsqueeze(1).to_broadcast((128, OWN, 1)))
                for k in range(OWN):
                    h2p = s2ps.tile([128, C + 1], f32, tag="h2p")
                    for kc in range(2):
                        nc.tensor.matmul(
                            h2p[:], z1Tl[:, kc, k * 128:(k + 1) * 128],
                            W2sb[:, kc, :], start=(kc == 0), stop=(kc == 1))
                    nc.vector.tensor_copy(h2l[:, k, 0:C], h2p[:, 0:C])
                    nc.vector.tensor_copy(h2l[:, k, C + 1:C + 2],
                                          h2p[:, C:C + 1])
                    nc.sync.dma_start(ag3_in[k * 128:(k + 1) * 128, :],
                                      h2l[:, k, :])

            nc.gpsimd.collective_compute(
                "AllGather", Alu.bypass,
                replica_groups=[list(range(NCORES))],
                ins=[ag3_in[:].opt()], outs=[ag3_out[:].opt()])
            ag3v = ag3_out[:].rearrange("(jc p) c -> p jc c", p=128)
            nc.sync.dma_start(h2all[:], ag3v)
            nc.vector.tensor_copy(s2dst[:], h2all[:, :, C + 1:C + 2])

            # ---------------- layer 2: masked softmax + aggregation -------
            with tc.tile_pool(name="aggps2", bufs=1, space="PSUM") as aggps2:
                o2 = aggps2.tile([AUGH, SH], f32)
                with tc.tile_pool(name="work2", bufs=6) as wp2:
                    for jc in range(NB):
                        pex = wp2.tile([128, SH], bf16, tag="pexb")
                        nc.scalar.activation(pex[:], s2srcb[:], Af.Exp,
                                             bias=s2dst[:, jc, :])
                        pt = wp2.tile([128, SH], bf16, tag="ptb")
                        nc.vector.tensor_mul(pt[:], pex[:], maskr[:, jc, :])
                        nc.tensor.matmul(o2[:], h2all[:, jc, 0:C + 1], pt[:],
                                         start=(jc == 0), stop=(jc == NB - 1))

                with tc.tile_pool(name="fin2", bufs=1) as fin2:
                    u2 = fin2.tile([1, SH], f32, tag="u2")
                    nc.scalar.activation(u2[:], o2[D1:D1 + 1, :], Af.Ln,
                                         scale=KREC)
                    nc.scalar.activation(rec2row[:], u2[:], Af.Exp, scale=-5.0)
                    recb2 = fin2.tile([C, SH], f32, tag="recb2")
                    nc.gpsimd.partition_broadcast(recb2[:], rec2row[:])
                    outsb = fin2.tile([C, SH], f32, tag="outsb")
                    nc.vector.scalar_tensor_tensor(
                        outsb[:], o2[0:D1, :], KREC, recb2[:],
                        op0=Alu.mult, op1=Alu.mult)
                    nc.sync.dma_start(outT_d, outsb[:])

    nc.compile()
    return nc


def _get_nc():
    if "nc" not in _CACHED:
        _CACHED["nc"] = _build_nc()
    return _CACHED["nc"]


def _prep_in_maps(x, A, W1, a1_src, a1_dst, W2, a2_src, a2_dst):
    import ml_dtypes
    f = np.float32
    bf = ml_dtypes.bfloat16
    xT = np.ascontiguousarray(x.T).astype(f, copy=False)
    W1r = W1.reshape(FIN, H, D1)
    V1s = np.einsum("fhd,hd->fh", W1r, a1_src).astype(f)
    V1d = np.einsum("fhd,hd->fh", W1r, a1_dst).astype(f)
    s_src = (x @ V1s).astype(f)                    # [N, H]
    s_dst = (x @ V1d).astype(f)                    # [N, H]
    E1s = np.exp(s_src)
    E2s = np.exp(NEG * s_src)
    qd_full = np.exp(-(1.0 - NEG) * s_dst)
    E1d_full = np.exp(s_dst)

    def jlay(a):                                   # [N, H] -> [128, NB, H]
        return np.ascontiguousarray(
            a.reshape(NB, 128, H).transpose(1, 0, 2)).astype(f)

    sdstT = jlay(s_dst)
    qdT = jlay(qd_full)
    W2e = np.concatenate([W2, W2 @ a2_dst.T], axis=1).astype(bf)
    v2s = (W2 @ a2_src.T).astype(bf)

    in_maps = []
    for c in range(NCORES):
        sl = slice(c * SH, (c + 1) * SH)

        def ibc(a, dt):                  # [SH, H] rows -> [128, H, SH] bcast
            r = np.ascontiguousarray(a[sl].T)      # [H, SH]
            return np.ascontiguousarray(
                np.broadcast_to(r[None], (128, H, SH))).astype(dt)

        # E1d for the k=3 own block only (nodes 512c+384 .. 512c+512)
        E1dloc = np.ascontiguousarray(
            E1d_full[c * SH + 384:c * SH + 512][None].transpose(
                1, 0, 2)).astype(f)
        in_maps.append({
            "xsT": np.ascontiguousarray(xT[:, sl]),
            "maskT": np.ascontiguousarray((A[sl, :] > 0).T).astype(bf),
            "W1a": W1.astype(f, copy=False),
            "ssrcb": ibc(s_src, bf),
            "E1sb": ibc(E1s, bf),
            "E2sb": ibc(E2s, bf),
            "sdstT": sdstT,
            "qdT": qdT,
            "E1dloc": E1dloc,
            "W2e": W2e,
            "v2s": v2s,
        })
    return in_maps


def kernel(x, A, W1, a1_src, a1_dst, W2, a2_src, a2_dst, _want_results=False):
    from concourse.bass_utils import run_bass_kernel_spmd

    nc = _get_nc()
    in_maps = _prep_in_maps(np.asarray(x), np.asarray(A), np.asarray(W1),
                            np.asarray(a1_src), np.asarray(a1_dst),
                            np.asarray(W2), np.asarray(a2_src),
                            np.asarray(a2_dst))
    trace = bool(int(os.environ.get("GAT_TRACE", "0")))
    res = run_bass_kernel_spmd(nc, in_maps, core_ids=list(range(NCORES)),
                               trace=trace)
    out = np.empty((N, C), np.float32)
    for c in range(NCORES):
        out[c * SH:(c + 1) * SH, :] = res.results[c]["outT"].T
    if _want_results:
        return out, res
    return out


# revision 8
# speedup vs baseline: 1.3572x; 1.0416x over previous
"""GAT (2-layer graph attention network) Bass kernel for 8 trn2 NeuronCores.

Sharding: core c owns node rows [512c, 512c+512). Each core projects only its
own 512 nodes (h = x_own @ W1) and all-gathers the augmented per-head blocks;
attention exponentials are computed per j-chunk in transposed layout
[j(partitions), i(free)].

Per-chunk paths (chosen by jc % 4, so the all-gather payload is one block per
chunk) share one PSUM accumulator per head:
  jc%4<3 (ACT path): P = patchedExp(s_src[i] + s_dst[j]) * mask
  jc%4=3 (MAX path): P = max(E2s[i]*qd[j], E1s[i])*E1d[j] * mask
               via exp(lrelu(t)) = max(exp(t), exp(0.2 t)), with exp(s_dst)
               folded into the gathered stationary block (htil).
Layer 2 projects h2 for the local node shard only (from local z1 columns) and
all-gathers the small augmented [h2 | 1 | s2_dst] blocks. The patched ACT exp
table computes exp(lrelu(x)); tables needing a true exp (E1s/E2s/qd/E1d) are
host-side folds of the rank-1 score projections x @ (W1 a1_*). Softmax
reciprocals run as ln/exp on the scalar engine (rec = patchedExp(-5*ln(K*den))
= 1/(K*den)); Exp is pinned to the natural_log_exp_and_others table set so
Exp/Ln share one table load. Both layer-1 phases are emitted before their
epilogues so no in-order engine stream stalls on a PSUM-gated epilogue op
between phases.
"""

import os

import numpy as np

N, FIN, HID, H, D1, C = 4096, 512, 256, 4, 64, 64
NCORES = 8
SH = N // NCORES          # 512 local nodes per core
NB = N // 128             # 32 j-chunks
FC = FIN // 128           # 4 fin chunks
OWN = 4                   # own j-blocks per core
NEG = 0.2
AUGH = D1 + 1             # 65 per head
AUG = AUGH * H            # 260
KREC = 32.0               # reciprocal pre-scale (keeps ln(K*den) in (0, 17))

_CACHED = {}


def _make_act_root(alpha=NEG):
    """Patch the neuron ACT tables so Exp computes g(x)=exp(lrelu(x)).

    Bucket entries are [d0,d1,d2,d3,x0,0,0,0] fp32 cubics evaluated as
    y = d0+(x-x0)(d1+(x-x0)(d2+(x-x0)d3)). For exp buckets centered at
    x0<0 we substitute the Taylor cubic of exp(alpha*x) at the same
    center. Ln buckets are untouched.
    """
    import json
    import shutil
    import tempfile

    from neuronxcc.driver.Job import Job
    from neuronxcc.driver.jobs.support.FindActInfo import findActInfoFile

    src_dir = os.path.dirname(findActInfoFile(Job.getPackageDir(), "gen3"))
    dst = tempfile.mkdtemp(prefix="gat_act_root_")
    for f in os.listdir(src_dir):
        shutil.copy(os.path.join(src_dir, f), os.path.join(dst, f))
        os.chmod(os.path.join(dst, f), 0o644)
    for set_name in ("exp_and_others", "natural_log_exp_and_others",
                     "exp_and_friends"):
        meta = json.load(open(os.path.join(dst, f"{set_name}.json")))
        start = meta["func_to_bkt_start_idx"].get("exp")
        if start is None:
            continue
        nxt = [s for s in sorted(meta["func_to_bkt_start_idx"].values())
               if s > start]
        end = nxt[0] if nxt else meta["bkt_entry_cnt"]
        path = os.path.join(dst, f"{set_name}_bkt.bin")
        b = np.fromfile(path, dtype=np.float32).reshape(-1, 8).copy()
        for i in range(start, end):
            x0, d0 = float(b[i, 4]), float(b[i, 0])
            if x0 >= 0 or not np.isfinite(d0) or d0 <= 0:
                continue
            e = np.exp(alpha * x0)
            b[i, 0:4] = [e, alpha * e, alpha * alpha * e / 2.0,
                         alpha ** 3 * e / 6.0]
        b.tofile(path)
    return os.path.join(dst, "act_info.json")


def _pin_exp_table(act_root, mybir, bacc):
    """Make bacc's table-load pass see Exp only in the set that also holds
    Ln, so the whole kernel uses one ACT table load (no Exp<->Ln flips)."""
    import json

    with open(act_root) as f:
        info = json.load(f)
    tables = {}
    for ent in info["act_func_sets"]:
        fns = set()
        for v in ent["act"].keys():
            try:
                fns.add(mybir.ActivationFunctionType.from_pwp(v))
            except Exception:
                pass
        if ent["name"] != "natural_log_exp_and_others":
            fns.discard(mybir.ActivationFunctionType.Exp)
        tables[ent["name"]] = fns
    bacc.get_activation_tables = lambda arch: tables


def _build_nc():
    act_root = _make_act_root()
    os.environ["BASS_ACT_ROOT_JSON_PATH"] = act_root
    import concourse.mybir as mybir
    import concourse.tile as tile
    from concourse import bacc

    _pin_exp_table(act_root, mybir, bacc)

    f32 = mybir.dt.float32
    f32r = mybir.dt.float32r
    bf16 = mybir.dt.bfloat16
    Af = mybir.ActivationFunctionType
    Alu = mybir.AluOpType

    nc = bacc.Bacc("TRN2", target_bir_lowering=False, debug=False,
                   num_devices=NCORES)

    xsT_d = nc.dram_tensor("xsT", [FIN, SH], f32r, kind="ExternalInput").ap()
    mT_d = nc.dram_tensor("maskT", [N, SH], bf16, kind="ExternalInput").ap()
    W1_d = nc.dram_tensor("W1a", [FIN, HID], f32r, kind="ExternalInput").ap()
    ssrcb_d = nc.dram_tensor("ssrcb", [128, H, SH], bf16, kind="ExternalInput").ap()
    E1sb_d = nc.dram_tensor("E1sb", [128, H, SH], bf16, kind="ExternalInput").ap()
    E2sb_d = nc.dram_tensor("E2sb", [128, H, SH], bf16, kind="ExternalInput").ap()
    sdst_d = nc.dram_tensor("sdstT", [128, NB, H], f32, kind="ExternalInput").ap()
    qd_d = nc.dram_tensor("qdT", [128, NB, H], f32, kind="ExternalInput").ap()
    E1dl_d = nc.dram_tensor("E1dloc", [128, 1, H], f32, kind="ExternalInput").ap()
    W2e_d = nc.dram_tensor("W2e", [HID, C + 1], bf16, kind="ExternalInput").ap()
    v2s_d = nc.dram_tensor("v2s", [HID, 1], bf16, kind="ExternalInput").ap()
    outT_d = nc.dram_tensor("outT", [C, SH], f32, kind="ExternalOutput").ap()

    with tile.TileContext(nc) as tc:
        with (tc.tile_pool(name="persist", bufs=1) as pp,
              tc.tile_pool(name="dram", bufs=1, space="DRAM") as dpool):
            # ---------------- persistent SBUF tiles -----------------------
            maskr = pp.tile([128, NB, SH], bf16)
            h1all = pp.tile([128, NB, AUG], bf16)
            ssrcb = pp.tile([128, H, SH], bf16)
            E1sb = pp.tile([128, H, SH], bf16)
            E2sb = pp.tile([128, H, SH], bf16)
            sdstT = pp.tile([128, NB, H], f32)
            qdT = pp.tile([128, NB, H], f32)
            E1dloc = pp.tile([128, 1, H], f32)
            xsTt = pp.tile([128, FC, SH], f32r)
            W1sb = pp.tile([128, FC, HID], f32r)
            h1loc = pp.tile([128, OWN, AUG], bf16)
            htloc = pp.tile([128, 1, AUG], bf16)
            z1Tl = pp.tile([128, 2, SH], bf16)
            h2l = pp.tile([128, OWN, C + 1], bf16)      # [h2 | 1]
            h2all = pp.tile([128, NB, C + 1], bf16)
            s2dst = pp.tile([128, NB, 1], f32)
            s2dcol = pp.tile([128, OWN], f32)
            pexf = pp.tile([128, NB, SH], bf16)
            s2srow = pp.tile([1, SH], f32)
            s2srcb = pp.tile([128, SH], f32)
            W2sb = pp.tile([128, 2, C + 1], bf16)
            v2sb = pp.tile([128, 2, 1], bf16)
            ones_col = pp.tile([128, 1], bf16)
            rec2row = pp.tile([1, SH], f32)

            ag1_in = dpool.tile([OWN * 128, AUG], bf16)
            ag1_out = dpool.tile([N, AUG], bf16, addr_space="Shared")
            ag3_in = dpool.tile([OWN * 128, C + 1], bf16)
            ag3_out = dpool.tile([N, C + 1], bf16, addr_space="Shared")
            ag4_in = dpool.tile([128, OWN], f32)
            ag4_out = dpool.tile([128 * NCORES, OWN], f32, addr_space="Shared")

            # ---------------- AG1-critical input DMAs ---------------------
            for fc in range(FC):
                nc.sync.dma_start(xsTt[:, fc, :], xsT_d[fc * 128:(fc + 1) * 128, :])
                nc.sync.dma_start(W1sb[:, fc, :], W1_d[fc * 128:(fc + 1) * 128, :])
            nc.sync.dma_start(E1dloc[:], E1dl_d)
            nc.vector.memset(ones_col[:], 1.0)

            # ---------------- local prep: h1aug (+ htil for k=3) ----------
            with tc.tile_pool(name="ppsum", bufs=2, space="PSUM") as ppsum:
                h1v = h1loc[:].rearrange("p k (h x) -> p k h x", x=AUGH)
                nc.vector.tensor_copy(
                    h1v[:, :, :, D1:D1 + 1],
                    ones_col[:].unsqueeze(1).unsqueeze(1).to_broadcast(
                        (128, OWN, H, 1)))
                for k in range(OWN):
                    hp = ppsum.tile([128, HID], f32, tag="hp")
                    for fc in range(FC):
                        nc.tensor.matmul(
                            hp[:], xsTt[:, fc, k * 128:(k + 1) * 128],
                            W1sb[:, fc, :],
                            start=(fc == 0), stop=(fc == FC - 1))
                    nc.vector.tensor_copy(
                        h1v[:, k, :, 0:D1],
                        hp[:].rearrange("p (h d) -> p h d", h=H))
                    if k == OWN - 1:
                        for h in range(H):
                            nc.vector.tensor_scalar_mul(
                                htloc[:, 0, h * AUGH:(h + 1) * AUGH],
                                h1loc[:, k, h * AUGH:(h + 1) * AUGH],
                                E1dloc[:, 0, h:h + 1])
                        nc.sync.dma_start(ag1_in[k * 128:(k + 1) * 128, :],
                                          htloc[:, 0, :])
                    else:
                        nc.sync.dma_start(ag1_in[k * 128:(k + 1) * 128, :],
                                          h1loc[:, k, :])

            nc.gpsimd.collective_compute(
                "AllGather", Alu.bypass,
                replica_groups=[list(range(NCORES))],
                ins=[ag1_in[:].opt()], outs=[ag1_out[:].opt()])

            # stream score tables during the all-gather
            nc.sync.dma_start(ssrcb[:], ssrcb_d)
            nc.sync.dma_start(sdstT[:], sdst_d)
            nc.sync.dma_start(E1sb[:], E1sb_d)
            nc.sync.dma_start(E2sb[:], E2sb_d)
            nc.sync.dma_start(qdT[:], qd_d)

            # h1all loads depend on AG1; mask/W2 loads are issued from the
            # gpsimd stream, which blocks on the collective completion, so
            # the big mask traffic stays off HBM until AG1 is done on every
            # core (it otherwise starves the slowest core's critical DMAs).
            ag1v = ag1_out[:].rearrange("(jc p) c -> p jc c", p=128)
            nc.sync.dma_start(h1all[:, 0:16, :], ag1v[:, 0:16, :])
            nc.sync.dma_start(h1all[:, 16:NB, :], ag1v[:, 16:NB, :])
            for jc in range(NB):
                nc.gpsimd.dma_start(maskr[:, jc, :],
                                    mT_d[jc * 128:(jc + 1) * 128, :])
            for kc in range(2):
                nc.gpsimd.dma_start(W2sb[:, kc, :],
                                    W2e_d[kc * 128:(kc + 1) * 128, :])
                nc.gpsimd.dma_start(v2sb[:, kc, :],
                                    v2s_d[kc * 128:(kc + 1) * 128, :])

            # ---------------- layer 1: two head-phases --------------------
            with tc.tile_pool(name="l1ps", bufs=1, space="PSUM") as l1ps:
                o1A = l1ps.tile([AUGH, 2, SH], f32, tag="o1A")
                o1B = l1ps.tile([AUGH, 2, SH], f32, tag="o1B")

                def emit_epilogue(ph, o1):
                    with tc.tile_pool(name=f"fin{ph}", bufs=1) as fin:
                        lnv = fin.tile([1, 2, SH], f32, tag="lnv")
                        nc.scalar.activation(lnv[:], o1[D1:D1 + 1, :, :],
                                             Af.Ln, scale=KREC)
                        rr = fin.tile([1, 2, SH], f32, tag="rr")
                        nc.scalar.activation(rr[:], lnv[:], Af.Exp, scale=-5.0)
                        zrow = fin.tile([128, SH], f32, tag="zrow")
                        for u in range(2):
                            recb = fin.tile([D1, SH], f32, tag=f"recb{u}")
                            nc.gpsimd.partition_broadcast(recb[:], rr[:, u, :])
                            nc.vector.scalar_tensor_tensor(
                                zrow[u * D1:(u + 1) * D1, :], o1[0:D1, u, :],
                                KREC, recb[:], op0=Alu.mult, op1=Alu.mult)
                        # ELU: max(z,0) + patchedExp(5*min(z,0)) - 1
                        rmax = fin.tile([128, SH], f32, tag="rmax")
                        rmin = fin.tile([128, SH], f32, tag="rmin")
                        ex = fin.tile([128, SH], f32, tag="ex")
                        nc.vector.tensor_scalar_max(rmax[:], zrow[:], 0.0)
                        nc.vector.tensor_scalar_min(rmin[:], zrow[:], 0.0)
                        nc.scalar.activation(ex[:], rmin[:], Af.Exp, scale=5.0)
                        nc.vector.scalar_tensor_tensor(
                            z1Tl[:, ph, :], ex[:], -1.0, rmax[:],
                            op0=Alu.add, op1=Alu.add)

                with tc.tile_pool(name="work", bufs=6) as wp:
                    for ph, o1 in ((0, o1A), (1, o1B)):
                        hs = [2 * ph, 2 * ph + 1]
                        for t in range(NB):
                            if ph == 1 and t == 7:
                                emit_epilogue(0, o1A)
                            jc = t
                            mb = maskr[:, jc, :].unsqueeze(1).to_broadcast(
                                (128, 2, SH))
                            if t % 4 != 3:
                                pex = wp.tile([128, 2, SH], bf16, tag="e0")
                                for u, h in enumerate(hs):
                                    nc.scalar.activation(
                                        pex[:, u, :], ssrcb[:, h, :], Af.Exp,
                                        bias=sdstT[:, jc, h:h + 1])
                                pt = wp.tile([128, 2, SH], bf16, tag="e2")
                                nc.vector.tensor_mul(pt[:], pex[:], mb)
                                src = pt
                            else:
                                t0 = wp.tile([128, 2, SH], bf16, tag="e0")
                                for u, h in enumerate(hs):
                                    nc.vector.tensor_scalar_mul(
                                        t0[:, u, :], E2sb[:, h, :],
                                        qdT[:, jc, h:h + 1])
                                t1 = wp.tile([128, 2, SH], bf16, tag="e1")
                                for u, h in enumerate(hs):
                                    nc.vector.tensor_max(
                                        t1[:, u, :], t0[:, u, :], E1sb[:, h, :])
                                m1 = wp.tile([128, 2, SH], bf16, tag="e2")
                                nc.vector.tensor_mul(m1[:], t1[:], mb)
                                src = m1
                            for u, h in enumerate(hs):
                                nc.tensor.matmul(
                                    o1[:, u, :],
                                    h1all[:, jc, AUGH * h:AUGH * (h + 1)],
                                    src[:, u, :],
                                    start=(t == 0), stop=(t == NB - 1))

                emit_epilogue(1, o1B)

            # ---------------- layer 2: local h2 projection + all-gather ---
            with tc.tile_pool(name="s2ps", bufs=2, space="PSUM") as s2ps:
                s2p = s2ps.tile([1, SH], f32, tag="s2p", bufs=1)
                for kc in range(2):
                    nc.tensor.matmul(s2p[:], v2sb[:, kc, :], z1Tl[:, kc, :],
                                     start=(kc == 0), stop=(kc == 1))
                nc.vector.tensor_copy(s2srow[:], s2p[:])
                nc.gpsimd.partition_broadcast(s2srcb[:], s2srow[:])

                # s2_dst for the local shard, in node-partition layout
                # ([p, k] columns); tiny all-gather so the layer-2
                # exponentials can precompute during the h2 gather.
                s2dp = s2ps.tile([128, OWN], f32, tag="s2dp", bufs=1)
                for k in range(OWN):
                    for kc in range(2):
                        nc.tensor.matmul(
                            s2dp[:, k:k + 1],
                            z1Tl[:, kc, k * 128:(k + 1) * 128],
                            W2sb[:, kc, C:C + 1],
                            start=(kc == 0), stop=(kc == 1))
                nc.vector.tensor_copy(s2dcol[:], s2dp[:])
                nc.sync.dma_start(ag4_in[:], s2dcol[:])
                nc.gpsimd.collective_compute(
                    "AllGather", Alu.bypass,
                    replica_groups=[list(range(NCORES))],
                    ins=[ag4_in[:].opt()], outs=[ag4_out[:].opt()])
                ag4v = ag4_out[:].rearrange("(r p) k -> p r k", p=128)
                s2dv = s2dst[:].rearrange("p (r k) x -> p r (k x)", k=OWN)
                nc.sync.dma_start(s2dv, ag4v)
                for jc in range(NB):
                    nc.scalar.activation(pexf[:, jc, :], s2srcb[:], Af.Exp,
                                         bias=s2dst[:, jc, :])

                nc.vector.tensor_copy(
                    h2l[:, :, C:C + 1],
                    ones_col[:].unsqueeze(1).to_broadcast((128, OWN, 1)))
                for k in range(OWN):
                    h2p = s2ps.tile([128, C + 1], f32, tag="h2p")
                    for kc in range(2):
                        nc.tensor.matmul(
                            h2p[:], z1Tl[:, kc, k * 128:(k + 1) * 128],
                            W2sb[:, kc, :], start=(kc == 0), stop=(kc == 1))
                    nc.vector.tensor_copy(h2l[:, k, 0:C], h2p[:, 0:C])
                    nc.sync.dma_start(ag3_in[k * 128:(k + 1) * 128, :],
                                      h2l[:, k, :])

            nc.gpsimd.collective_compute(
                "AllGather", Alu.bypass,
                replica_groups=[list(range(NCORES))],
                ins=[ag3_in[:].opt()], outs=[ag3_out[:].opt()])
            ag3v = ag3_out[:].rearrange("(jc p) c -> p jc c", p=128)
            nc.sync.dma_start(h2all[:], ag3v)

            # ---------------- layer 2: masked softmax + aggregation -------
            with tc.tile_pool(name="aggps2", bufs=1, space="PSUM") as aggps2:
                o2 = aggps2.tile([AUGH, SH], f32)
                with tc.tile_pool(name="work2", bufs=16) as wp2:
                    for jc in range(NB):
                        pt = wp2.tile([128, SH], bf16, tag="ptb")
                        nc.vector.tensor_mul(pt[:], pexf[:, jc, :],
                                             maskr[:, jc, :])
                        nc.tensor.matmul(o2[:], h2all[:, jc, :], pt[:],
                                         start=(jc == 0), stop=(jc == NB - 1))

                with tc.tile_pool(name="fin2", bufs=1) as fin2:
                    u2 = fin2.tile([1, SH], f32, tag="u2")
                    nc.scalar.activation(u2[:], o2[D1:D1 + 1, :], Af.Ln,
                                         scale=KREC)
                    nc.scalar.activation(rec2row[:], u2[:], Af.Exp, scale=-5.0)
                    recb2 = fin2.tile([C, SH], f32, tag="recb2")
                    nc.gpsimd.partition_broadcast(recb2[:], rec2row[:])
                    outsb = fin2.tile([C, SH], f32, tag="outsb")
                    nc.vector.scalar_tensor_tensor(
                        outsb[:], o2[0:D1, :], KREC, recb2[:],
                        op0=Alu.mult, op1=Alu.mult)
                    nc.sync.dma_start(outT_d, outsb[:])

    nc.compile()
    return nc


def _get_nc():
    if "nc" not in _CACHED:
        _CACHED["nc"] = _build_nc()
    return _CACHED["nc"]


def _prep_in_maps(x, A, W1, a1_src, a1_dst, W2, a2_src, a2_dst):
    import ml_dtypes
    f = np.float32
    bf = ml_dtypes.bfloat16
    xT = np.ascontiguousarray(x.T).astype(f, copy=False)
    W1r = W1.reshape(FIN, H, D1)
    V1s = np.einsum("fhd,hd->fh", W1r, a1_src).astype(f)
    V1d = np.einsum("fhd,hd->fh", W1r, a1_dst).astype(f)
    s_src = (x @ V1s).astype(f)                    # [N, H]
    s_dst = (x @ V1d).astype(f)                    # [N, H]
    E1s = np.exp(s_src)
    E2s = np.exp(NEG * s_src)
    qd_full = np.exp(-(1.0 - NEG) * s_dst)
    E1d_full = np.exp(s_dst)

    def jlay(a):                                   # [N, H] -> [128, NB, H]
        return np.ascontiguousarray(
            a.reshape(NB, 128, H).transpose(1, 0, 2)).astype(f)

    sdstT = jlay(s_dst)
    qdT = jlay(qd_full)
    W2e = np.concatenate([W2, W2 @ a2_dst.T], axis=1).astype(bf)
    v2s = (W2 @ a2_src.T).astype(bf)

    in_maps = []
    for c in range(NCORES):
        sl = slice(c * SH, (c + 1) * SH)

        def ibc(a, dt):                  # [SH, H] rows -> [128, H, SH] bcast
            r = np.ascontiguousarray(a[sl].T)      # [H, SH]
            return np.ascontiguousarray(
                np.broadcast_to(r[None], (128, H, SH))).astype(dt)

        # E1d for the k=3 own block only (nodes 512c+384 .. 512c+512)
        E1dloc = np.ascontiguousarray(
            E1d_full[c * SH + 384:c * SH + 512][None].transpose(
                1, 0, 2)).astype(f)
        in_maps.append({
            "xsT": np.ascontiguousarray(xT[:, sl]),
            "maskT": np.ascontiguousarray((A[sl, :] > 0).T).astype(bf),
            "W1a": W1.astype(f, copy=False),
            "ssrcb": ibc(s_src, bf),
            "E1sb": ibc(E1s, bf),
            "E2sb": ibc(E2s, bf),
            "sdstT": sdstT,
            "qdT": qdT,
            "E1dloc": E1dloc,
            "W2e": W2e,
            "v2s": v2s,
        })
    return in_maps


def kernel(x, A, W1, a1_src, a1_dst, W2, a2_src, a2_dst, _want_results=False):
    from concourse.bass_utils import run_bass_kernel_spmd

    nc = _get_nc()
    in_maps = _prep_in_maps(np.asarray(x), np.asarray(A), np.asarray(W1),
                            np.asarray(a1_src), np.asarray(a1_dst),
                            np.asarray(W2), np.asarray(a2_src),
                            np.asarray(a2_dst))
    trace = bool(int(os.environ.get("GAT_TRACE", "0")))
    res = run_bass_kernel_spmd(nc, in_maps, core_ids=list(range(NCORES)),
                               trace=trace)
    out = np.empty((N, C), np.float32)
    for c in range(NCORES):
        out[c * SH:(c + 1) * SH, :] = res.results[c]["outT"].T
    if _want_results:
        return out, res
    return out
